# revision 1
# baseline (speedup 1.0000x reference)
"""Multi-head attention (B=2, S=2048, D=2048, H=16) on 8 Trainium2 cores.

Sharding: core = batch (2) x head-group (4 heads each). Tensor-parallel on
wq/wk/wv rows + wo columns; per-core partial outputs summed on host.

Device kernel (per core, all matmuls in float32r):
  phase 1: qT/kT (head_dim, seq) + v (seq, head_dim) projections, RoPE on q/k
  phase 2: scores^T -> exp -> denominator (ones-matmul) + attn@V, normalize
  phase 3: output projection partial (seq, dim)
"""

import sys

for _p in ("/opt/trn_rl_repo",):
    if _p not in sys.path:
        sys.path.insert(0, _p)

import numpy as np

import concourse.bass as bass
import concourse.tile as tile
from concourse import bacc, mybir
from concourse.bass_utils import run_bass_kernel_spmd

F32 = mybir.dt.float32
F32R = mybir.dt.float32r

DIM = 2048
N_HEADS = 16
HEAD_DIM = 128
BATCH = 2
SEQ = 2048
G_HEADS = 4          # heads per core
GM = G_HEADS * HEAD_DIM  # 512 output cols per core
DC = DIM // 128      # 16 contraction chunks
SC512 = SEQ // 512   # 4
SC128 = SEQ // 128   # 16
INV_SQRT_HD = float(1.0 / np.sqrt(HEAD_DIM))

# even<->odd partition swap within each 32-partition group
_SWAP_MASK = [i ^ 1 for i in range(32)]


def build(with_mask: bool):
    nc = bacc.Bacc("TRN2", target_bir_lowering=False, debug=False)

    xt_d = nc.dram_tensor("xt", [DC, 128, SEQ], F32R, kind="ExternalInput").ap()
    wq_d = nc.dram_tensor("wq", [DC, G_HEADS, 128, 128], F32R, kind="ExternalInput").ap()
    wk_d = nc.dram_tensor("wk", [DC, G_HEADS, 128, 128], F32R, kind="ExternalInput").ap()
    wv_d = nc.dram_tensor("wv", [DC, 128, GM], F32R, kind="ExternalInput").ap()
    wo_d = nc.dram_tensor("wo", [G_HEADS, 128, SEQ], F32R, kind="ExternalInput").ap()
    ce_d = nc.dram_tensor("ce", [128, SEQ], F32R, kind="ExternalInput").ap()
    s2_d = nc.dram_tensor("s2", [128, SEQ], F32R, kind="ExternalInput").ap()
    ones_d = nc.dram_tensor("ones", [128, 1], F32R, kind="ExternalInput").ap()
    mt_d = None
    if with_mask:
        mt_d = nc.dram_tensor("mt", [SC128, 128, SEQ], F32, kind="ExternalInput").ap()
    out_d = nc.dram_tensor("out", [SC128, 128, SEQ], F32, kind="ExternalOutput").ap()

    with tile.TileContext(nc) as tc:
        with (
            tc.tile_pool(name="persist", bufs=1) as persist,
            tc.tile_pool(name="consts", bufs=1) as consts,
        ):
            ones_t = consts.tile([128, 1], F32R, tag="ones")
            nc.sync.dma_start(ones_t[:], ones_d)
            # warm the ACT exp LUT early so phase 2 doesn't stall on it
            warm_t = consts.tile([128, 1], F32, tag="warm")
            nc.scalar.activation(
                out=warm_t[:], in_=ones_t[:],
                func=mybir.ActivationFunctionType.Exp,
            )

            q_t = [persist.tile([128, SEQ], F32R, tag=f"q{h}", name=f"q{h}") for h in range(G_HEADS)]
            k_t = [persist.tile([128, SEQ], F32R, tag=f"k{h}", name=f"k{h}") for h in range(G_HEADS)]
            v_t = [persist.tile([128, GM], F32R, tag=f"v{s}", name=f"v{s}") for s in range(SC128)]

            # ---------------- phase 1: projections + fused rope ----------------
            with (
                tc.tile_pool(name="rope_c", bufs=1) as rope_c,
                tc.tile_pool(name="xt", bufs=8) as xt_pool,
                tc.tile_pool(name="wqk", bufs=2) as wqk_pool,
                tc.tile_pool(name="wv", bufs=1) as wv_pool,
                tc.tile_pool(name="ps1", bufs=4, space="PSUM") as ps1,
                tc.tile_pool(name="rope_t", bufs=1) as rope_t,
            ):
                ce_t = rope_c.tile([128, SEQ], F32R, tag="ce")
                s2_t = rope_c.tile([128, SEQ], F32R, tag="s2")

                def rope(t, sl):
                    # sin-product on the otherwise-idle GpSimd engine
                    t1 = rope_t.tile([128, 512], F32, tag="t1", name="t1")
                    nc.gpsimd.tensor_mul(out=t1[:], in0=t[:, sl], in1=s2_t[:, sl])
                    t2 = rope_t.tile([128, 512], F32, tag="t2", name="t2")
                    nc.vector.stream_shuffle(t2[:], t1[:], _SWAP_MASK)
                    t3 = rope_t.tile([128, 512], F32, tag="t3", name="t3")
                    nc.vector.tensor_mul(out=t3[:], in0=t[:, sl], in1=ce_t[:, sl])
                    nc.vector.tensor_add(out=t[:, sl], in0=t3[:], in1=t2[:])

                for half in range(2):
                    dcs = list(range(half * 8, half * 8 + 8))
                    # first head's weights + first-half x tiles are what the
                    # very first matmul chain needs: pin them to the front of
                    # the scheduler's priority heap so no other dep-free DMA
                    # (wvt/ce/s2/...) gets hoisted ahead of them
                    prio = tc.high_priority() if half == 0 else None
                    if prio is not None:
                        prio.__enter__()
                    wt_first = wqk_pool.tile([128, 8, 128], F32R, tag="w", name="wt")
                    nc.sync.dma_start(
                        wt_first[:],
                        wq_d[dcs[0] : dcs[0] + 8, 0].rearrange("c p m -> p c m"),
                    )
                    # x tiles split into s-halves (separate tiles so the
                    # first chains unblock after half the data), alternating
                    # issue queues (SP / ACT) for parallel DMA
                    xtsA, xtsB = [], []
                    for qi, dc in enumerate(dcs):
                        xa = xt_pool.tile([128, 1024], F32R, tag="xa", name="xa")
                        eng = nc.sync if qi % 2 == 0 else nc.scalar
                        eng.dma_start(xa[:], xt_d[dc][:, 0:1024])
                        xtsA.append(xa)
                    if prio is not None:
                        prio.__exit__(None, None, None)
                    for qi, dc in enumerate(dcs):
                        xb = xt_pool.tile([128, 1024], F32R, tag="xb", name="xb")
                        eng = nc.sync if qi % 2 == 1 else nc.scalar
                        eng.dma_start(xb[:], xt_d[dc][:, 1024:2048])
                        xtsB.append(xb)

                    def xslice(i, sl_start, width):
                        # rhs slice [sl_start, sl_start+width) of logical xt[i]
                        if sl_start < 1024:
                            return xtsA[i][:, sl_start : sl_start + width]
                        return xtsB[i][:, sl_start - 1024 : sl_start - 1024 + width]
                    wvt = wv_pool.tile([128, 8, GM], F32R, tag="wv", name="wvt")
                    nc.scalar.dma_start(
                        wvt[:], wv_d[dcs[0] : dcs[0] + 8].rearrange("c p m -> p c m")
                    )
                    v_next = 0

                    def emit_v(n):
                        nonlocal v_next
                        for s in range(v_next, v_next + n):
                            ps = ps1.tile([128, GM], F32, tag="ps", name="ps")
                            for i in range(8):
                                nc.tensor.matmul(
                                    ps[:], xslice(i, s * 128, 128), wvt[:, i, :],
                                    start=(i == 0), stop=(i == 7),
                                )
                            if half == 0:
                                nc.vector.tensor_copy(out=v_t[s][:], in_=ps[:])
                            else:
                                nc.vector.tensor_add(
                                    out=v_t[s][:], in0=ps[:], in1=v_t[s][:]
                                )
                        v_next += n

                    # v-group placement: back-loaded in half 0 (wvt DMA queues
                    # behind the xt bulk); interleaved finely in half 1 so DVE
                    # rope work never outpaces PE for long
                    for h in range(G_HEADS):
                        for wi, (wd, dst) in enumerate(((wq_d, q_t[h]), (wk_d, k_t[h]))):
                            if h == 0 and wi == 0:
                                wt = wt_first
                            else:
                                wt = wqk_pool.tile([128, 8, 128], F32R, tag="w", name="wt")
                                nc.sync.dma_start(
                                    wt[:],
                                    wd[dcs[0] : dcs[0] + 8, h].rearrange("c p m -> p c m"),
                                )
                            for sc in range(SC512):
                                ps = ps1.tile([128, 512], F32, tag="ps", name="ps")
                                sl = bass.ts(sc, 512)
                                for i in range(8):
                                    nc.tensor.matmul(
                                        ps[:], wt[:, i, :], xslice(i, sc * 512, 512),
                                        start=(i == 0), stop=(i == 7),
                                    )
                                if half == 0:
                                    nc.vector.tensor_copy(out=dst[:, sl], in_=ps[:])
                                else:
                                    nc.vector.tensor_add(
                                        out=dst[:, sl], in0=ps[:], in1=dst[:, sl]
                                    )
                                    rope(dst, sl)
                        if (half == 0 and h >= 2) or half == 1:
                            emit_v({0: 8, 1: 4}[half])
                        if half == 0 and h == G_HEADS - 1:
                            # rope tables: needed from half 1 on
                            nc.scalar.dma_start(ce_t[:], ce_d)
                            nc.scalar.dma_start(s2_t[:], s2_d)

            # ---------------- phase 2: attention ----------------
            o_pool = tc.alloc_tile_pool(name="oT", bufs=1)
            o_t = [o_pool.tile([128, SEQ], F32R, tag=f"o{h}", name=f"o{h}") for h in range(G_HEADS)]
            wo_pool = tc.alloc_tile_pool(name="wo", bufs=1)
            wo_t = []
            for mc in range(G_HEADS):
                w = wo_pool.tile([128, SEQ], F32R, tag=f"wo{mc}", name=f"wo{mc}")
                nc.sync.dma_start(w[:], wo_d[mc])
                wo_t.append(w)
            with (
                tc.tile_pool(name="est", bufs=5) as est_pool,
                tc.tile_pool(name="nrm", bufs=3) as nrm_pool,
                tc.tile_pool(name="ps_st", bufs=2, space="PSUM") as ps_st,
                tc.tile_pool(name="ps_av", bufs=2, space="PSUM") as ps_av,
                tc.tile_pool(name="ps_dn", bufs=2, space="PSUM") as ps_dn,
            ):
                if with_mask:
                    mask_pool = tc.alloc_tile_pool(name="mask", bufs=2)

                for ic in range(SC512):
                    isl = bass.ts(ic, 512)
                    for hp in range(G_HEADS // 2):
                        heads = (2 * hp, 2 * hp + 1)
                        acc = {}
                        den = {}
                        e_of = {}
                        m_of = {}
                        for h in heads:
                            acc[h] = ps_av.tile([128, 512], F32, tag="acc", name="acc")
                            den[h] = ps_dn.tile([1, 512], F32, tag="den", name="den")

                        def emit_st(h, jc2):
                            ja, jb = 2 * jc2, 2 * jc2 + 1
                            st = ps_st.tile([128, 1024], F32, tag="st", name="st")
                            nc.tensor.matmul(
                                st[:, 0:512],
                                k_t[h][:, bass.ts(ja, 128)], q_t[h][:, isl],
                                start=True, stop=True,
                            )
                            nc.tensor.matmul(
                                st[:, 512:1024],
                                k_t[h][:, bass.ts(jb, 128)], q_t[h][:, isl],
                                start=True, stop=True,
                            )
                            e = est_pool.tile([128, 1024], F32R, tag="e", name="e")
                            if with_mask:
                                if jc2 not in m_of:
                                    mtl = mask_pool.tile(
                                        [128, 1024], F32, tag="m", name="mtl"
                                    )
                                    nc.sync.dma_start(mtl[:, 0:512], mt_d[ja, :, isl])
                                    nc.sync.dma_start(mtl[:, 512:1024], mt_d[jb, :, isl])
                                    m_of[jc2] = mtl
                                nc.vector.tensor_add(
                                    out=e[:], in0=st[:], in1=m_of[jc2][:]
                                )
                                nc.scalar.activation(
                                    out=e[:], in_=e[:],
                                    func=mybir.ActivationFunctionType.Exp,
                                )
                            else:
                                nc.scalar.activation(
                                    out=e[:], in_=st[:],
                                    func=mybir.ActivationFunctionType.Exp,
                                )
                            # pre-sum the two j-chunk halves on DVE so the
                            # denominator needs one PE matmul, not two
                            eh = est_pool.tile([128, 512], F32R, tag="eh", name="eh", bufs=3)
                            nc.vector.tensor_add(
                                out=eh[:], in0=e[:, 0:512], in1=e[:, 512:1024]
                            )
                            e_of[h] = (e, eh)

                        def emit_denav(h, jc2):
                            ja, jb = 2 * jc2, 2 * jc2 + 1
                            e, eh = e_of[h]
                            last = jc2 == SC128 // 2 - 1
                            nc.tensor.matmul(
                                den[h][:], ones_t[:], eh[:],
                                start=(jc2 == 0), stop=last,
                            )
                            nc.tensor.matmul(
                                acc[h][:], v_t[ja][:, bass.ts(h, 128)], e[:, 0:512],
                                start=(jc2 == 0), stop=False,
                            )
                            nc.tensor.matmul(
                                acc[h][:], v_t[jb][:, bass.ts(h, 128)], e[:, 512:1024],
                                start=False, stop=last,
                            )


                        # software pipeline: heads offset by a half step so PE
                        # always has independent matmuls while ACT runs exp
                        h0, h1 = heads
                        NJ2 = SC128 // 2
                        emit_st(h0, 0)
                        for jc2 in range(NJ2):
                            if jc2 > 0:
                                emit_st(h0, jc2)
                                emit_denav(h1, jc2 - 1)
                            emit_st(h1, jc2)
                            emit_denav(h0, jc2)
                        emit_denav(h1, NJ2 - 1)

                        for h in heads:
                            rec = nrm_pool.tile([1, 512], F32, tag="rec", name="rec")
                            nc.vector.reciprocal_approx_fast(out=rec[:], in_=den[h][:])
                            bc = nrm_pool.tile([128, 512], F32, tag="bc", name="bc")
                            nc.gpsimd.partition_broadcast(bc[:], rec[:])
                            nc.vector.tensor_mul(
                                out=o_t[h][:, isl], in0=acc[h][:], in1=bc[:]
                            )
                if with_mask:
                    mask_pool.release()

            # ---------------- phase 3: output projection ----------------
            with (
                tc.tile_pool(name="fin", bufs=10) as fin_pool,
                tc.tile_pool(name="ps3", bufs=8, space="PSUM") as ps3,
            ):
                for s in range(SC128):
                    ssl = bass.ts(s, 128)
                    for nck in range(SC512):
                        nsl = bass.ts(nck, 512)
                        ps = ps3.tile([128, 512], F32, tag="ps3", name="ps3")
                        for mc in range(G_HEADS):
                            nc.tensor.matmul(
                                ps[:], o_t[mc][:, ssl], wo_t[mc][:, nsl],
                                start=(mc == 0), stop=(mc == G_HEADS - 1),
                            )
                        f = fin_pool.tile([128, 512], F32, tag="f", name="f")
                        nc.vector.tensor_copy(out=f[:], in_=ps[:])
                        nc.sync.dma_start(out_d[s, :, nsl], f[:])
            wo_pool.release()
            o_pool.release()

    nc.compile()
    return nc


_CACHE = {}


def _get_nc(with_mask: bool):
    if with_mask not in _CACHE:
        _CACHE[with_mask] = build(with_mask)
    return _CACHE[with_mask]


def kernel(in_token, freqs_cos, freqs_sin, mask, wq, wk, wv, wo):
    return _run(in_token, freqs_cos, freqs_sin, mask, wq, wk, wv, wo)


def run_traced(in_token, freqs_cos, freqs_sin, mask, wq, wk, wv, wo):
    """Test-only: run with NTFF tracing, return (output, BassKernelResults)."""
    return _run(in_token, freqs_cos, freqs_sin, mask, wq, wk, wv, wo, trace=True)


def _run(in_token, freqs_cos, freqs_sin, mask, wq, wk, wv, wo, trace=False):
    in_token = np.ascontiguousarray(np.asarray(in_token, dtype=np.float32))
    freqs_cos = np.asarray(freqs_cos, dtype=np.float32)
    freqs_sin = np.asarray(freqs_sin, dtype=np.float32)
    mask = np.asarray(mask, dtype=np.float32)
    wq = np.asarray(wq, dtype=np.float32)
    wk = np.asarray(wk, dtype=np.float32)
    wv = np.asarray(wv, dtype=np.float32)
    wo = np.asarray(wo, dtype=np.float32)

    with_mask = bool(np.any(mask))
    nc = _get_nc(with_mask)

    # rope tables in (head_dim, seq) pair-expanded layout, signs/swap baked in
    ce = np.repeat(freqs_cos.T, 2, axis=0).astype(np.float32)  # (128, S)
    s2 = np.empty((HEAD_DIM, SEQ), np.float32)
    s2[0::2] = freqs_sin.T   # even rows: +sin (lands on odd out after swap)
    s2[1::2] = -freqs_sin.T  # odd rows: -sin (lands on even out after swap)
    ones = np.ones((128, 1), np.float32)
    if with_mask:
        mt = np.ascontiguousarray(mask.T).reshape(SC128, 128, SEQ)

    in_maps = []
    xts = [
        np.ascontiguousarray(in_token[b].T).reshape(DC, 128, SEQ)
        for b in range(BATCH)
    ]
    for b in range(BATCH):
        for g in range(G_HEADS):
            rows = slice(g * GM, (g + 1) * GM)
            wqt = np.ascontiguousarray(
                (wq[rows] * INV_SQRT_HD).T.reshape(
                    DC, 128, G_HEADS, 128
                ).transpose(0, 2, 1, 3)
            )
            wkt = np.ascontiguousarray(
                wk[rows].T.reshape(DC, 128, G_HEADS, 128).transpose(0, 2, 1, 3)
            )
            wvt = np.ascontiguousarray(wv[rows].T).reshape(DC, 128, GM)
            wot = np.ascontiguousarray(wo[:, rows].T).reshape(G_HEADS, 128, SEQ)
            m = {
                "xt": xts[b], "wq": wqt, "wk": wkt, "wv": wvt, "wo": wot,
                "ce": ce, "s2": s2, "ones": ones,
            }
            if with_mask:
                m["mt"] = mt
            in_maps.append(m)

    res = run_bass_kernel_spmd(nc, in_maps, core_ids=list(range(8)), trace=trace)

    out = np.zeros((BATCH, SEQ, DIM), np.float32)
    for b in range(BATCH):
        acc = None
        for g in range(G_HEADS):
            p = res.results[b * G_HEADS + g]["out"].reshape(SEQ, DIM)
            acc = p if acc is None else acc + p
        out[b] = acc
    if trace:
        return out, res
    return out



# revision 36
# speedup vs baseline: 1.2368x; 1.2368x over previous
"""Multi-head attention (B=2, S=2048, D=2048, H=16) on 8 Trainium2 cores.

Sharding: core = batch (2) x head-group (4 heads each). Tensor-parallel on
wq/wk/wv rows + wo columns; per-core partial outputs summed on host.

Fast (mask-free) build: all matmul operands bf16 (same PE rate as fp32r in
the TRN2 cost model, half the DMA/SBUF, 2x DVE modes), one-pass QKV
projections with 16-chunk PSUM accumulation, softmax denominator via bf16
DVE accumulation + a single ones-matmul per (ic, head), h2/h3 projections
woven into the first attention blocks and the output projection woven into
the second ones so the PE never waits on the exp pipeline.

Masked build: previous two-pass fp32r kernel (correct, slower; unused when
mask is all zeros).
"""

import sys

for _p in ("/opt/trn_rl_repo",):
    if _p not in sys.path:
        sys.path.insert(0, _p)

import numpy as np
import ml_dtypes

import concourse.bass as bass
import concourse.bass_isa as bass_isa
import concourse.tile as tile
from concourse import bacc, mybir
from concourse.bass_utils import run_bass_kernel_spmd

F32 = mybir.dt.float32
F32R = mybir.dt.float32r
BF16 = mybir.dt.bfloat16
NPBF16 = ml_dtypes.bfloat16
EXP = mybir.ActivationFunctionType.Exp
ACT_COPY = mybir.ActivationFunctionType.Copy

DIM = 2048
N_HEADS = 16
HEAD_DIM = 128
BATCH = 2
SEQ = 2048
G_HEADS = 4          # heads per core
GM = G_HEADS * HEAD_DIM  # 512 output cols per core
DC = DIM // 128      # 16 contraction chunks
SC512 = SEQ // 512   # 4
SC128 = SEQ // 128   # 16
NJ2 = SC128 // 2     # 8 j-chunk pairs
INV_SQRT_HD = float(1.0 / np.sqrt(HEAD_DIM))

# even<->odd partition swap within each 32-partition group
_SWAP_MASK = [i ^ 1 for i in range(32)]


def build_fast():
    nc = bacc.Bacc("TRN2", target_bir_lowering=False, debug=False)

    xt_d = nc.dram_tensor("xt", [DC, 128, SEQ], BF16, kind="ExternalInput").ap()
    wq_d = nc.dram_tensor("wq", [G_HEADS, 128, DC * 128], BF16, kind="ExternalInput").ap()
    wk_d = nc.dram_tensor("wk", [G_HEADS, 128, DC * 128], BF16, kind="ExternalInput").ap()
    wv_d = nc.dram_tensor("wv", [128, DC * GM], BF16, kind="ExternalInput").ap()
    wo_d = nc.dram_tensor("wo", [G_HEADS, 128, SEQ], BF16, kind="ExternalInput").ap()
    ce_d = nc.dram_tensor("ce", [128, SEQ], BF16, kind="ExternalInput").ap()
    s2_d = nc.dram_tensor("s2", [128, SEQ], BF16, kind="ExternalInput").ap()
    ones_d = nc.dram_tensor("ones", [128, 1], BF16, kind="ExternalInput").ap()
    out_d = nc.dram_tensor("out", [SC128, 128, SEQ], BF16, kind="ExternalOutput").ap()

    with tile.TileContext(nc) as tc:
        consts = tc.alloc_tile_pool(name="consts", bufs=1)
        persist = tc.alloc_tile_pool(name="persist", bufs=1)
        rope_t = tc.alloc_tile_pool(name="ropet", bufs=2)

        q_t = [persist.tile([128, SEQ], BF16, tag=f"q{h}", name=f"q{h}") for h in range(G_HEADS)]
        k_t = [persist.tile([128, SEQ], BF16, tag=f"k{h}", name=f"k{h}") for h in range(G_HEADS)]
        v_t = [persist.tile([128, GM], BF16, tag=f"v{s}", name=f"v{s}") for s in range(SC128)]
        ce_t = persist.tile([128, SEQ], BF16, tag="ce", name="ce")
        s2_t = persist.tile([128, SEQ], BF16, tag="s2", name="s2")

        xf_pool = tc.alloc_tile_pool(name="xf", bufs=1)
        wqk_pool = tc.alloc_tile_pool(name="wqk", bufs=3)
        wv_pool = tc.alloc_tile_pool(name="wvp", bufs=1)
        ps_proj = tc.alloc_tile_pool(name="psproj", bufs=8, space="PSUM")

        # ---------- startup DMAs ----------
        xf = [xf_pool.tile([128, SEQ], BF16, tag=f"x{dc}", name=f"x{dc}") for dc in range(DC)]
        w0q = wqk_pool.tile([128, DC * 128], BF16, tag="w", name="w0q")
        w0k = wqk_pool.tile([128, DC * 128], BF16, tag="w", name="w0k")
        ones_t = consts.tile([128, 1], BF16, tag="ones", name="ones")
        junk_t = consts.tile([128, 512], BF16, tag="junk", name="junk")
        with tc.high_priority():
            nc.sync.dma_start(xf[0][:, 0:1024], xt_d[0][:, 0:1024])
            nc.sync.dma_start(w0q[:, 0:512], wq_d[0][:, 0:512])
            nc.scalar.dma_start(xf[0][:, 1024:], xt_d[0][:, 1024:])
            nc.scalar.dma_start(w0k[:, 0:512], wk_d[0][:, 0:512])
            nc.gpsimd.dma_start(ones_t[:], ones_d)
            # PE p-state warm-up: a few throwaway matmuls so the ramp epoch
            # starts at ~1.5us and the first real matmuls run at full rate
            nc.vector.memset(junk_t[:], 0.5)
            warm_ps = ps_proj.tile([128, 512], F32, tag="pp", name="warm_ps")
            for i in range(5):
                nc.tensor.matmul(warm_ps[:], junk_t[:, 0:128], junk_t[:],
                                 start=(i == 0), stop=(i == 4))
        warm_t = consts.tile([128, 1], F32, tag="warm", name="warm")
        nc.scalar.activation(out=warm_t[:], in_=ones_t[:], func=EXP)
        for dc in range(1, 4):
            eng = nc.scalar if dc % 2 == 1 else nc.sync
            eng.dma_start(xf[dc][:], xt_d[dc])
        nc.sync.dma_start(w0q[:, 512:], wq_d[0][:, 512:])
        nc.scalar.dma_start(w0k[:, 512:], wk_d[0][:, 512:])
        for dc in range(4, DC):
            eng = nc.scalar if dc % 2 == 1 else nc.sync
            eng.dma_start(xf[dc][:], xt_d[dc])
        # h1 weights then rope tables — all needed right at the end of G0.
        # w1k reuses w0q's pool slot (frees at G0 end) so it must not sit
        # ahead of ce/s2 in a queue.
        w1q = wqk_pool.tile([128, DC * 128], BF16, tag="w", name="w1q")
        nc.sync.dma_start(w1q[:], wq_d[1])
        w1k = wqk_pool.tile([128, DC * 128], BF16, tag="w", name="w1k")
        nc.sync.dma_start(w1k[:], wk_d[1])
        nc.scalar.dma_start(ce_t[:], ce_d)
        nc.scalar.dma_start(s2_t[:], s2_d)

        def rope_split(ps, dst, sc, t3_pool=False):
            """ACT-copy the PSUM chain result into its bf16 slice, start the
            sin-product on GpSimd; return a closure emitting the remaining
            DVE ops (all-SBUF bf16, so they run in the 2x mode). GPSIMD must
            never touch PSUM (BIR verifier rule)."""
            sl = slice(sc * 512, (sc + 1) * 512)
            nc.scalar.activation(out=dst[:, sl], in_=ps[:], func=ACT_COPY)
            t1 = rope_t.tile([128, 512], BF16, tag="t1", name="t1")
            nc.gpsimd.tensor_mul(out=t1[:], in0=dst[:, sl], in1=s2_t[:, sl])

            def back():
                t3 = rope_t.tile([128, 512], BF16, tag="t3", name="t3")
                nc.vector.tensor_mul(out=t3[:], in0=dst[:, sl], in1=ce_t[:, sl])
                t2 = rope_t.tile([128, 512], BF16, tag="t2", name="t2")
                nc.vector.stream_shuffle(t2[:], t1[:], _SWAP_MASK)
                nc.vector.tensor_add(out=dst[:, sl], in0=t3[:], in1=t2[:])
            return back

        def rope(ps, dst, sc, t3_pool=False):
            rope_split(ps, dst, sc, t3_pool)()

        # ---------- G0: h0 q,k — dc-major, paced by x arrival ----------
        chains0 = [("q", sc) for sc in range(SC512)] + [("k", sc) for sc in range(SC512)]
        ps0 = [ps_proj.tile([128, 512], F32, tag="pp", name=f"pp{ci}")
               for ci in range(len(chains0))]
        for dc in range(DC):
            for ci, (qk, sc) in enumerate(chains0):
                w = w0q if qk == "q" else w0k
                nc.tensor.matmul(
                    ps0[ci][:], w[:, dc * 128:(dc + 1) * 128],
                    xf[dc][:, sc * 512:(sc + 1) * 512],
                    start=(dc == 0), stop=(dc == DC - 1),
                )
        for ci, (qk, sc) in enumerate(chains0):
            rope(ps0[ci], q_t[0] if qk == "q" else k_t[0], sc)

        # ---------- G1: h1 q,k — chain-major ----------
        for qk, w, dst in (("q", w1q, q_t[1]), ("k", w1k, k_t[1])):
            for sc in range(SC512):
                ps = ps_proj.tile([128, 512], F32, tag="pp", name="pp")
                for dc in range(DC):
                    nc.tensor.matmul(
                        ps[:], w[:, dc * 128:(dc + 1) * 128],
                        xf[dc][:, sc * 512:(sc + 1) * 512],
                        start=(dc == 0), stop=(dc == DC - 1),
                    )
                rope(ps, dst, sc)

        # ---------- V: 16 chains, ACT copies to v tiles ----------
        wvt = wv_pool.tile([128, DC * GM], BF16, tag="wv", name="wvt")
        for i in range(4):
            nc.scalar.dma_start(
                wvt[:, i * 2048:(i + 1) * 2048], wv_d[:, i * 2048:(i + 1) * 2048]
            )
        for s in range(SC128):
            ps = ps_proj.tile([128, GM], F32, tag="pp", name="pp")
            for dc in range(DC):
                nc.tensor.matmul(
                    ps[:], xf[dc][:, s * 128:(s + 1) * 128],
                    wvt[:, dc * GM:(dc + 1) * GM],
                    start=(dc == 0), stop=(dc == DC - 1),
                )
            nc.scalar.activation(out=v_t[s][:], in_=ps[:], func=ACT_COPY)
        wv_pool.release()

        # weights for the h2/h3 fill chains (k first: needed by hp1 earliest)
        w2k = wqk_pool.tile([128, DC * 128], BF16, tag="w", name="w2k")
        nc.sync.dma_start(w2k[:], wk_d[2])
        w3k = wqk_pool.tile([128, DC * 128], BF16, tag="w", name="w3k")
        nc.scalar.dma_start(w3k[:], wk_d[3])
        w2q = wqk_pool.tile([128, DC * 128], BF16, tag="w", name="w2q")
        nc.sync.dma_start(w2q[:], wq_d[2])
        w3q = wqk_pool.tile([128, DC * 128], BF16, tag="w", name="w3q")
        nc.scalar.dma_start(w3q[:], wq_d[3])

        ps_proj.release()

        # ---------- attention pools ----------
        o_pool = tc.alloc_tile_pool(name="oT", bufs=1)
        o_t = [o_pool.tile([128, SEQ], BF16, tag=f"o{h}", name=f"o{h}") for h in range(G_HEADS)]
        e_pool = tc.alloc_tile_pool(name="e", bufs=4)
        acc_pool = tc.alloc_tile_pool(name="acce", bufs=3)
        eh_pool = tc.alloc_tile_pool(name="eh", bufs=2)
        red_pool = tc.alloc_tile_pool(name="red", bufs=3)
        f_pool = tc.alloc_tile_pool(name="fp", bufs=4)
        ps_st = tc.alloc_tile_pool(name="ps_st", bufs=2, space="PSUM")
        ps_av = tc.alloc_tile_pool(name="ps_av", bufs=2, space="PSUM")
        ps_fill = tc.alloc_tile_pool(name="ps_fill", bufs=2, space="PSUM")

        wo_pool = tc.alloc_tile_pool(name="wo", bufs=1)
        wo_t = []  # tiles created after xf releases (SBUF headroom)

        def p3_mm(s, nck, psum_pool):
            ps = psum_pool.tile([128, 512], F32, tag="fill", name="psf")
            for mc in range(G_HEADS):
                nc.tensor.matmul(
                    ps[:], o_t[mc][:, s * 128:(s + 1) * 128],
                    wo_t[mc][:, nck * 512:(nck + 1) * 512],
                    start=(mc == 0), stop=(mc == G_HEADS - 1),
                )
            return ps

        def p3_out(s, nck, ps, eng, fstate):
            # pairs of 512-col results share one [128,1024] staging tile and
            # one store, halving HWDGE issue pressure
            if nck % 2 == 0:
                fstate["f"] = f_pool.tile([128, 1024], BF16, tag="f", name="f")
            fo = fstate["f"]
            half = (nck % 2) * 512
            dst = fo[:, half:half + 512]
            if eng == 0:
                nc.scalar.activation(out=dst, in_=ps[:], func=ACT_COPY)
            else:
                nc.vector.tensor_copy(out=dst, in_=ps[:])
            if nck % 2 == 1:
                st_eng = nc.scalar if (nck // 2) % 2 == 0 else nc.sync
                st_eng.dma_start(out_d[s][:, (nck - 1) * 512:(nck + 1) * 512], fo[:])

        def make_g23_fill(chains):
            # h2/h3 projection chains, 4 matmul-groups each. The rope-front
            # consumers of chain i-1 are emitted after chain i's first group
            # so the two ps_fill banks cycle without stalling PE.
            state = {"i": 0, "g": 0, "ps": None,
                     "front": None, "back": None, "done": False}

            def flush_front():
                if state["front"] is not None:
                    state["back"] = state["front"]()
                    state["front"] = None

            def flush_back():
                if state["back"] is not None:
                    state["back"]()
                    state["back"] = None

            def fill():
                i = state["i"]
                if i >= len(chains):
                    if state["front"] or state["back"]:
                        flush_front()
                        flush_back()
                        return True
                    return False
                h, qk, sc, w = chains[i]
                g = state["g"]
                if g == 0:
                    state["ps"] = ps_fill.tile([128, 512], F32, tag="fill", name="psf")
                ps = state["ps"]
                for dc in range(4 * g, 4 * g + 4):
                    nc.tensor.matmul(
                        ps[:], w[:, dc * 128:(dc + 1) * 128],
                        xf[dc][:, sc * 512:(sc + 1) * 512],
                        start=(dc == 0), stop=(dc == DC - 1),
                    )
                if g == 0:
                    flush_front()
                elif g == 1:
                    flush_back()
                elif g == 3:
                    dst = q_t[h] if qk == "q" else k_t[h]
                    state["front"] = (
                        lambda p=ps, d=dst, s_=sc: rope_split(p, d, s_, t3_pool=True)
                    )
                    state["i"] += 1
                state["g"] = (g + 1) % 4
                return True
            return fill

        def make_p3_fill(ic, skip=4, psum_pool=None, engs=(0, 1, 0, 1)):
            # chain n's copy+store lag one slot behind its matmuls
            pool = psum_pool if psum_pool is not None else ps_fill
            state = {"n": -skip, "prev": None, "f": None}

            def fill():
                n = state["n"]
                if n >= 16:
                    if state["prev"] is not None:
                        s, nck, ps, k = state["prev"]
                        p3_out(s, nck, ps, engs[k % len(engs)], state)
                        state["prev"] = None
                        return True
                    return False
                state["n"] += 1
                if n < 0:
                    return True
                s, nck = ic * 4 + n // 4, n % 4
                ps = p3_mm(s, nck, pool)
                if state["prev"] is not None:
                    s_, nck_, ps_, k_ = state["prev"]
                    p3_out(s_, nck_, ps_, engs[k_ % len(engs)], state)
                state["prev"] = (s, nck, ps, n)
                return True
            return fill

        def noop_fill():
            return False

        def emit_block(ha, hb, ic, fill, prev_tail):
            """One attention block for local heads (ha, hb) on query chunk ic.
            prev_tail: (eh_fns, den_fns, fin_fns) per-head closures of the
            previous block's denominator/normalize, woven in here so the PE
            never waits on them at a block seam. Returns this block's tail."""
            isl = slice(ic * 512, (ic + 1) * 512)
            acc = {}
            acc_e = {}
            e_of = {}
            for h in (ha, hb):
                acc[h] = ps_av.tile([128, 512], F32, tag="acc", name="acc")

            def emit_st(h, jc2, copy_pool=False):
                ja, jb = 2 * jc2, 2 * jc2 + 1
                st = ps_st.tile([128, 1024], F32, tag="st", name="st")
                nc.tensor.matmul(
                    st[:, 0:512], k_t[h][:, ja * 128:(ja + 1) * 128], q_t[h][:, isl],
                    start=True, stop=True,
                )
                nc.tensor.matmul(
                    st[:, 512:1024], k_t[h][:, jb * 128:(jb + 1) * 128], q_t[h][:, isl],
                    start=True, stop=True,
                )
                e = e_pool.tile([128, 1024], BF16, tag="e", name="e")
                nc.scalar.activation(out=e[:], in_=st[:], func=EXP)
                if jc2 == 0:
                    a = acc_pool.tile([128, 1024], BF16, tag="acce", name="acce")
                    acc_e[h] = a
                    # first copy off the DVE queue head so the previous
                    # block's eh/recip chain runs unblocked
                    eng = nc.gpsimd if copy_pool else nc.vector
                    eng.tensor_copy(out=a[:], in_=e[:])
                else:
                    nc.vector.tensor_add(out=acc_e[h][:], in0=acc_e[h][:], in1=e[:])
                e_of[(h, jc2)] = e

            def emit_av(h, jc2):
                ja, jb = 2 * jc2, 2 * jc2 + 1
                e = e_of.pop((h, jc2))
                nc.tensor.matmul(
                    acc[h][:], v_t[ja][:, h * 128:(h + 1) * 128], e[:, 0:512],
                    start=(jc2 == 0), stop=False,
                )
                nc.tensor.matmul(
                    acc[h][:], v_t[jb][:, h * 128:(h + 1) * 128], e[:, 512:1024],
                    start=False, stop=(jc2 == NJ2 - 1),
                )

            p_eh, p_den, p_fin = prev_tail if prev_tail else (None, None, None)
            if p_eh:
                p_eh[0]()
            emit_st(ha, 0, copy_pool=True)
            if p_eh:
                p_eh[1]()
            fill()
            if p_den:
                p_den[0]()
            emit_st(hb, 0)
            fill()
            if p_den:
                p_den[1]()
            if p_fin:
                p_fin[0]()
                p_fin[1]()
            for jc2 in range(1, NJ2):
                emit_st(ha, jc2)
                fill()
                emit_av(ha, jc2 - 1)
                emit_st(hb, jc2)
                fill()
                emit_av(hb, jc2 - 1)
            emit_av(ha, NJ2 - 1)
            fill()
            emit_av(hb, NJ2 - 1)
            fill()

            # build this block's tail closures
            eh_of = {}
            red_of = {}

            def mk_eh(h):
                def f():
                    eh = eh_pool.tile([128, 512], BF16, tag="eh", name="eh")
                    nc.vector.tensor_add(
                        out=eh[:], in0=acc_e[h][:, 0:512], in1=acc_e[h][:, 512:1024]
                    )
                    eh_of[h] = eh
                return f

            def mk_red(h):
                def f():
                    # fused partition-reduce + broadcast of the softmax
                    # denominator on the otherwise-idle GpSimd engine
                    red = red_pool.tile([128, 512], F32, tag="red", name="red")
                    nc.gpsimd.partition_all_reduce(
                        red[:], eh_of[h][:], 128, bass_isa.ReduceOp.add
                    )
                    nc.vector.reciprocal_approx_fast(out=red[:], in_=red[:])
                    red_of[h] = red
                return f

            def mk_fin(h):
                def f():
                    nc.vector.tensor_mul(
                        out=o_t[h][:, isl], in0=acc[h][:], in1=red_of[h][:]
                    )
                return f

            return ([mk_eh(ha), mk_eh(hb)],
                    [mk_red(ha), mk_red(hb)],
                    [mk_fin(ha), mk_fin(hb)])

        def run_tail(t):
            eh_fns, den_fns, fin_fns = t
            for f in eh_fns:
                f()
            for f in den_fns:
                f()
            for f in fin_fns:
                f()

        # hp0: heads 0,1 with h2/h3 projection fill; the last two chains
        # (q2/q3 sc3, not needed until hp1-ic3) spill into hp1-ic0 so that
        # block has PE filler too
        g23_chains = []
        for qk, wmap in (("k", {2: w2k, 3: w3k}), ("q", {2: w2q, 3: w3q})):
            for h in (2, 3):
                for sc in range(SC512):
                    g23_chains.append((h, qk, sc, wmap[h]))
        g23 = make_g23_fill(g23_chains[:14])
        g23b = make_g23_fill(g23_chains[14:])
        tail = None
        for ic in range(SC512):
            tail = emit_block(0, 1, ic, g23, tail)
        while g23():
            pass

        for mc in range(G_HEADS):
            w = wo_pool.tile([128, SEQ], BF16, tag=f"wo{mc}", name=f"wo{mc}")
            eng = nc.sync if mc % 2 == 0 else nc.scalar
            eng.dma_start(w[:], wo_d[mc])
            wo_t.append(w)

        # hp1: heads 2,3 with output-projection fill (one ic behind)
        fills = [g23b] + [make_p3_fill(ic) for ic in range(SC512 - 1)]
        for ic in range(SC512):
            tail = emit_block(2, 3, ic, fills[ic], tail)
            if ic < SC512 - 1:
                while fills[ic]():
                    pass

        # leftover p3(ic2) chains cover the final tail's eh->den latency
        left = fills[SC512 - 1]
        left()
        run_tail(tail)
        while left():
            pass

        ps_fill.release()
        ps_av.release()
        ps_st.release()

        ps3 = tc.alloc_tile_pool(name="ps3", bufs=4, space="PSUM")
        tail_fill = make_p3_fill(SC512 - 1, skip=0, psum_pool=ps3,
                                 engs=(0, 1, 0, 1))
        while tail_fill():
            pass
        ps3.release()

        for p in (wo_pool, f_pool, red_pool, eh_pool, acc_pool,
                  e_pool, o_pool, wqk_pool, xf_pool, rope_t, persist, consts):
            p.release()

    nc.compile()
    return nc


def build_masked():
    """Previous two-pass fp32r kernel — only used when mask is nonzero."""
    nc = bacc.Bacc("TRN2", target_bir_lowering=False, debug=False)

    xt_d = nc.dram_tensor("xt", [DC, 128, SEQ], F32R, kind="ExternalInput").ap()
    wq_d = nc.dram_tensor("wq", [DC, G_HEADS, 128, 128], F32R, kind="ExternalInput").ap()
    wk_d = nc.dram_tensor("wk", [DC, G_HEADS, 128, 128], F32R, kind="ExternalInput").ap()
    wv_d = nc.dram_tensor("wv", [DC, 128, GM], F32R, kind="ExternalInput").ap()
    wo_d = nc.dram_tensor("wo", [G_HEADS, 128, SEQ], F32R, kind="ExternalInput").ap()
    ce_d = nc.dram_tensor("ce", [128, SEQ], F32R, kind="ExternalInput").ap()
    s2_d = nc.dram_tensor("s2", [128, SEQ], F32R, kind="ExternalInput").ap()
    ones_d = nc.dram_tensor("ones", [128, 1], F32R, kind="ExternalInput").ap()
    mt_d = nc.dram_tensor("mt", [SC128, 128, SEQ], F32, kind="ExternalInput").ap()
    out_d = nc.dram_tensor("out", [SC128, 128, SEQ], F32, kind="ExternalOutput").ap()

    with tile.TileContext(nc) as tc:
        with (
            tc.tile_pool(name="persist", bufs=1) as persist,
            tc.tile_pool(name="consts", bufs=1) as consts,
        ):
            ones_t = consts.tile([128, 1], F32R, tag="ones")
            nc.sync.dma_start(ones_t[:], ones_d)
            warm_t = consts.tile([128, 1], F32, tag="warm")
            nc.scalar.activation(
                out=warm_t[:], in_=ones_t[:],
                func=EXP,
            )

            q_t = [persist.tile([128, SEQ], F32R, tag=f"q{h}", name=f"q{h}") for h in range(G_HEADS)]
            k_t = [persist.tile([128, SEQ], F32R, tag=f"k{h}", name=f"k{h}") for h in range(G_HEADS)]
            v_t = [persist.tile([128, GM], F32R, tag=f"v{s}", name=f"v{s}") for s in range(SC128)]

            with (
                tc.tile_pool(name="rope_c", bufs=1) as rope_c,
                tc.tile_pool(name="xt", bufs=8) as xt_pool,
                tc.tile_pool(name="wqk", bufs=2) as wqk_pool,
                tc.tile_pool(name="wv", bufs=1) as wv_pool,
                tc.tile_pool(name="ps1", bufs=4, space="PSUM") as ps1,
                tc.tile_pool(name="rope_t", bufs=1) as rope_t,
            ):
                ce_t = rope_c.tile([128, SEQ], F32R, tag="ce")
                s2_t = rope_c.tile([128, SEQ], F32R, tag="s2")

                def rope(t, sl):
                    t1 = rope_t.tile([128, 512], F32, tag="t1", name="t1")
                    nc.gpsimd.tensor_mul(out=t1[:], in0=t[:, sl], in1=s2_t[:, sl])
                    t2 = rope_t.tile([128, 512], F32, tag="t2", name="t2")
                    nc.vector.stream_shuffle(t2[:], t1[:], _SWAP_MASK)
                    t3 = rope_t.tile([128, 512], F32, tag="t3", name="t3")
                    nc.vector.tensor_mul(out=t3[:], in0=t[:, sl], in1=ce_t[:, sl])
                    nc.vector.tensor_add(out=t[:, sl], in0=t3[:], in1=t2[:])

                for half in range(2):
                    dcs = list(range(half * 8, half * 8 + 8))
                    prio = tc.high_priority() if half == 0 else None
                    if prio is not None:
                        prio.__enter__()
                    wt_first = wqk_pool.tile([128, 8, 128], F32R, tag="w", name="wt")
                    nc.sync.dma_start(
                        wt_first[:],
                        wq_d[dcs[0] : dcs[0] + 8, 0].rearrange("c p m -> p c m"),
                    )
                    xtsA, xtsB = [], []
                    for qi, dc in enumerate(dcs):
                        xa = xt_pool.tile([128, 1024], F32R, tag="xa", name="xa")
                        eng = nc.sync if qi % 2 == 0 else nc.scalar
                        eng.dma_start(xa[:], xt_d[dc][:, 0:1024])
                        xtsA.append(xa)
                    if prio is not None:
                        prio.__exit__(None, None, None)
                    for qi, dc in enumerate(dcs):
                        xb = xt_pool.tile([128, 1024], F32R, tag="xb", name="xb")
                        eng = nc.sync if qi % 2 == 1 else nc.scalar
                        eng.dma_start(xb[:], xt_d[dc][:, 1024:2048])
                        xtsB.append(xb)

                    def xslice(i, sl_start, width):
                        if sl_start < 1024:
                            return xtsA[i][:, sl_start : sl_start + width]
                        return xtsB[i][:, sl_start - 1024 : sl_start - 1024 + width]
                    wvt = wv_pool.tile([128, 8, GM], F32R, tag="wv", name="wvt")
                    nc.scalar.dma_start(
                        wvt[:], wv_d[dcs[0] : dcs[0] + 8].rearrange("c p m -> p c m")
                    )
                    v_next = 0

                    def emit_v(n):
                        nonlocal v_next
                        for s in range(v_next, v_next + n):
                            ps = ps1.tile([128, GM], F32, tag="ps", name="ps")
                            for i in range(8):
                                nc.tensor.matmul(
                                    ps[:], xslice(i, s * 128, 128), wvt[:, i, :],
                                    start=(i == 0), stop=(i == 7),
                                )
                            if half == 0:
                                nc.vector.tensor_copy(out=v_t[s][:], in_=ps[:])
                            else:
                                nc.vector.tensor_add(
                                    out=v_t[s][:], in0=ps[:], in1=v_t[s][:]
                                )
                        v_next += n

                    for h in range(G_HEADS):
                        for wi, (wd, dst) in enumerate(((wq_d, q_t[h]), (wk_d, k_t[h]))):
                            if h == 0 and wi == 0:
                                wt = wt_first
                            else:
                                wt = wqk_pool.tile([128, 8, 128], F32R, tag="w", name="wt")
                                nc.sync.dma_start(
                                    wt[:],
                                    wd[dcs[0] : dcs[0] + 8, h].rearrange("c p m -> p c m"),
                                )
                            for sc in range(SC512):
                                ps = ps1.tile([128, 512], F32, tag="ps", name="ps")
                                sl = bass.ts(sc, 512)
                                for i in range(8):
                                    nc.tensor.matmul(
                                        ps[:], wt[:, i, :], xslice(i, sc * 512, 512),
                                        start=(i == 0), stop=(i == 7),
                                    )
                                if half == 0:
                                    nc.vector.tensor_copy(out=dst[:, sl], in_=ps[:])
                                else:
                                    nc.vector.tensor_add(
                                        out=dst[:, sl], in0=ps[:], in1=dst[:, sl]
                                    )
                                    rope(dst, sl)
                        if (half == 0 and h >= 2) or half == 1:
                            emit_v({0: 8, 1: 4}[half])
                        if half == 0 and h == G_HEADS - 1:
                            nc.scalar.dma_start(ce_t[:], ce_d)
                            nc.scalar.dma_start(s2_t[:], s2_d)

            o_pool = tc.alloc_tile_pool(name="oT", bufs=1)
            o_t = [o_pool.tile([128, SEQ], F32R, tag=f"o{h}", name=f"o{h}") for h in range(G_HEADS)]
            wo_pool = tc.alloc_tile_pool(name="wo", bufs=1)
            wo_t = []
            for mc in range(G_HEADS):
                w = wo_pool.tile([128, SEQ], F32R, tag=f"wo{mc}", name=f"wo{mc}")
                nc.sync.dma_start(w[:], wo_d[mc])
                wo_t.append(w)
            with (
                tc.tile_pool(name="est", bufs=5) as est_pool,
                tc.tile_pool(name="nrm", bufs=3) as nrm_pool,
                tc.tile_pool(name="ps_st", bufs=2, space="PSUM") as ps_st,
                tc.tile_pool(name="ps_av", bufs=2, space="PSUM") as ps_av,
                tc.tile_pool(name="ps_dn", bufs=2, space="PSUM") as ps_dn,
            ):
                mask_pool = tc.alloc_tile_pool(name="mask", bufs=2)

                for ic in range(SC512):
                    isl = bass.ts(ic, 512)
                    for hp in range(G_HEADS // 2):
                        heads = (2 * hp, 2 * hp + 1)
                        acc = {}
                        den = {}
                        e_of = {}
                        m_of = {}
                        for h in heads:
                            acc[h] = ps_av.tile([128, 512], F32, tag="acc", name="acc")
                            den[h] = ps_dn.tile([1, 512], F32, tag="den", name="den")

                        def emit_st(h, jc2):
                            ja, jb = 2 * jc2, 2 * jc2 + 1
                            st = ps_st.tile([128, 1024], F32, tag="st", name="st")
                            nc.tensor.matmul(
                                st[:, 0:512],
                                k_t[h][:, bass.ts(ja, 128)], q_t[h][:, isl],
                                start=True, stop=True,
                            )
                            nc.tensor.matmul(
                                st[:, 512:1024],
                                k_t[h][:, bass.ts(jb, 128)], q_t[h][:, isl],
                                start=True, stop=True,
                            )
                            e = est_pool.tile([128, 1024], F32R, tag="e", name="e")
                            if jc2 not in m_of:
                                mtl = mask_pool.tile(
                                    [128, 1024], F32, tag="m", name="mtl"
                                )
                                nc.sync.dma_start(mtl[:, 0:512], mt_d[ja, :, isl])
                                nc.sync.dma_start(mtl[:, 512:1024], mt_d[jb, :, isl])
                                m_of[jc2] = mtl
                            nc.vector.tensor_add(
                                out=e[:], in0=st[:], in1=m_of[jc2][:]
                            )
                            nc.scalar.activation(
                                out=e[:], in_=e[:],
                                func=EXP,
                            )
                            eh = est_pool.tile([128, 512], F32R, tag="eh", name="eh", bufs=3)
                            nc.vector.tensor_add(
                                out=eh[:], in0=e[:, 0:512], in1=e[:, 512:1024]
                            )
                            e_of[h] = (e, eh)

                        def emit_denav(h, jc2):
                            ja, jb = 2 * jc2, 2 * jc2 + 1
                            e, eh = e_of[h]
                            last = jc2 == SC128 // 2 - 1
                            nc.tensor.matmul(
                                den[h][:], ones_t[:], eh[:],
                                start=(jc2 == 0), stop=last,
                            )
                            nc.tensor.matmul(
                                acc[h][:], v_t[ja][:, bass.ts(h, 128)], e[:, 0:512],
                                start=(jc2 == 0), stop=False,
                            )
                            nc.tensor.matmul(
                                acc[h][:], v_t[jb][:, bass.ts(h, 128)], e[:, 512:1024],
                                start=False, stop=last,
                            )

                        h0, h1 = heads
                        emit_st(h0, 0)
                        for jc2 in range(NJ2):
                            if jc2 > 0:
                                emit_st(h0, jc2)
                                emit_denav(h1, jc2 - 1)
                            emit_st(h1, jc2)
                            emit_denav(h0, jc2)
                        emit_denav(h1, NJ2 - 1)

                        for h in heads:
                            rec = nrm_pool.tile([1, 512], F32, tag="rec", name="rec")
                            nc.vector.reciprocal_approx_fast(out=rec[:], in_=den[h][:])
                            bc = nrm_pool.tile([128, 512], F32, tag="bc", name="bc")
                            nc.gpsimd.partition_broadcast(bc[:], rec[:])
                            nc.vector.tensor_mul(
                                out=o_t[h][:, isl], in0=acc[h][:], in1=bc[:]
                            )
                mask_pool.release()

            with (
                tc.tile_pool(name="fin", bufs=10) as fin_pool,
                tc.tile_pool(name="ps3", bufs=8, space="PSUM") as ps3,
            ):
                for s in range(SC128):
                    ssl = bass.ts(s, 128)
                    for nck in range(SC512):
                        nsl = bass.ts(nck, 512)
                        ps = ps3.tile([128, 512], F32, tag="ps3", name="ps3")
                        for mc in range(G_HEADS):
                            nc.tensor.matmul(
                                ps[:], o_t[mc][:, ssl], wo_t[mc][:, nsl],
                                start=(mc == 0), stop=(mc == G_HEADS - 1),
                            )
                        f = fin_pool.tile([128, 512], F32, tag="f", name="f")
                        nc.vector.tensor_copy(out=f[:], in_=ps[:])
                        nc.sync.dma_start(out_d[s, :, nsl], f[:])
            wo_pool.release()
            o_pool.release()

    nc.compile()
    return nc


_CACHE = {}


def _get_nc(with_mask: bool):
    if with_mask not in _CACHE:
        _CACHE[with_mask] = build_masked() if with_mask else build_fast()
    return _CACHE[with_mask]


def kernel(in_token, freqs_cos, freqs_sin, mask, wq, wk, wv, wo):
    return _run(in_token, freqs_cos, freqs_sin, mask, wq, wk, wv, wo)


def run_traced(in_token, freqs_cos, freqs_sin, mask, wq, wk, wv, wo):
    """Test-only: run with NTFF tracing, return (output, BassKernelResults)."""
    return _run(in_token, freqs_cos, freqs_sin, mask, wq, wk, wv, wo, trace=True)


def _run(in_token, freqs_cos, freqs_sin, mask, wq, wk, wv, wo, trace=False):
    in_token = np.ascontiguousarray(np.asarray(in_token, dtype=np.float32))
    freqs_cos = np.asarray(freqs_cos, dtype=np.float32)
    freqs_sin = np.asarray(freqs_sin, dtype=np.float32)
    mask = np.asarray(mask, dtype=np.float32)
    wq = np.asarray(wq, dtype=np.float32)
    wk = np.asarray(wk, dtype=np.float32)
    wv = np.asarray(wv, dtype=np.float32)
    wo = np.asarray(wo, dtype=np.float32)

    with_mask = bool(np.any(mask))
    nc = _get_nc(with_mask)

    if with_mask:
        return _run_masked(nc, in_token, freqs_cos, freqs_sin, mask,
                           wq, wk, wv, wo, trace)

    ce = np.repeat(freqs_cos.T, 2, axis=0).astype(NPBF16)  # (128, S)
    s2 = np.empty((HEAD_DIM, SEQ), np.float32)
    s2[0::2] = freqs_sin.T
    s2[1::2] = -freqs_sin.T
    s2 = s2.astype(NPBF16)
    ones = np.ones((128, 1), NPBF16)

    in_maps = []
    xts = [
        np.ascontiguousarray(in_token[b].T).reshape(DC, 128, SEQ).astype(NPBF16)
        for b in range(BATCH)
    ]
    for b in range(BATCH):
        for g in range(G_HEADS):
            rows = slice(g * GM, (g + 1) * GM)
            wqt = np.ascontiguousarray(
                (wq[rows] * INV_SQRT_HD).T.reshape(DC, 128, G_HEADS, 128)
                .transpose(2, 1, 0, 3).reshape(G_HEADS, 128, DC * 128)
            ).astype(NPBF16)
            wkt = np.ascontiguousarray(
                wk[rows].T.reshape(DC, 128, G_HEADS, 128)
                .transpose(2, 1, 0, 3).reshape(G_HEADS, 128, DC * 128)
            ).astype(NPBF16)
            wvt = np.ascontiguousarray(
                wv[rows].T.reshape(DC, 128, GM).transpose(1, 0, 2)
                .reshape(128, DC * GM)
            ).astype(NPBF16)
            wot = np.ascontiguousarray(wo[:, rows].T).reshape(
                G_HEADS, 128, SEQ
            ).astype(NPBF16)
            m = {
                "xt": xts[b], "wq": wqt, "wk": wkt, "wv": wvt, "wo": wot,
                "ce": ce, "s2": s2, "ones": ones,
            }
            in_maps.append(m)

    res = run_bass_kernel_spmd(nc, in_maps, core_ids=list(range(8)), trace=trace)

    out = np.zeros((BATCH, SEQ, DIM), np.float32)
    for b in range(BATCH):
        acc = None
        for g in range(G_HEADS):
            p = res.results[b * G_HEADS + g]["out"].astype(np.float32).reshape(SEQ, DIM)
            acc = p if acc is None else acc + p
        out[b] = acc
    if trace:
        return out, res
    return out


def _run_masked(nc, in_token, freqs_cos, freqs_sin, mask, wq, wk, wv, wo, trace):
    ce = np.repeat(freqs_cos.T, 2, axis=0).astype(np.float32)  # (128, S)
    s2 = np.empty((HEAD_DIM, SEQ), np.float32)
    s2[0::2] = freqs_sin.T
    s2[1::2] = -freqs_sin.T
    ones = np.ones((128, 1), np.float32)
    mt = np.ascontiguousarray(mask.T).reshape(SC128, 128, SEQ)

    in_maps = []
    xts = [
        np.ascontiguousarray(in_token[b].T).reshape(DC, 128, SEQ)
        for b in range(BATCH)
    ]
    for b in range(BATCH):
        for g in range(G_HEADS):
            rows = slice(g * GM, (g + 1) * GM)
            wqt = np.ascontiguousarray(
                (wq[rows] * INV_SQRT_HD).T.reshape(
                    DC, 128, G_HEADS, 128
                ).transpose(0, 2, 1, 3)
            )
            wkt = np.ascontiguousarray(
                wk[rows].T.reshape(DC, 128, G_HEADS, 128).transpose(0, 2, 1, 3)
            )
            wvt = np.ascontiguousarray(wv[rows].T).reshape(DC, 128, GM)
            wot = np.ascontiguousarray(wo[:, rows].T).reshape(G_HEADS, 128, SEQ)
            m = {
                "xt": xts[b], "wq": wqt, "wk": wkt, "wv": wvt, "wo": wot,
                "ce": ce, "s2": s2, "ones": ones, "mt": mt,
            }
            in_maps.append(m)

    res = run_bass_kernel_spmd(nc, in_maps, core_ids=list(range(8)), trace=trace)

    out = np.zeros((BATCH, SEQ, DIM), np.float32)
    for b in range(BATCH):
        acc = None
        for g in range(G_HEADS):
            p = res.results[b * G_HEADS + g]["out"].reshape(SEQ, DIM)
            acc = p if acc is None else acc + p
        out[b] = acc
    if trace:
        return out, res
    return out


# revision 45
# speedup vs baseline: 1.2380x; 1.0010x over previous
"""Multi-head attention (B=2, S=2048, D=2048, H=16) on 8 Trainium2 cores.

Sharding: core = batch (2) x head-group (4 heads each). Tensor-parallel on
wq/wk/wv rows + wo columns; per-core partial outputs summed on host.

Fast (mask-free) build: all matmul operands bf16 (same PE rate as fp32r in
the TRN2 cost model, half the DMA/SBUF, 2x DVE modes), one-pass QKV
projections with 16-chunk PSUM accumulation, softmax denominator via bf16
DVE accumulation + a single ones-matmul per (ic, head), h2/h3 projections
woven into the first attention blocks and the output projection woven into
the second ones so the PE never waits on the exp pipeline.

Masked build: previous two-pass fp32r kernel (correct, slower; unused when
mask is all zeros).
"""

import sys

for _p in ("/opt/trn_rl_repo",):
    if _p not in sys.path:
        sys.path.insert(0, _p)

import numpy as np
import ml_dtypes

import concourse.bass as bass
import concourse.bass_isa as bass_isa
import concourse.tile as tile
from concourse import bacc, mybir
from concourse.bass_utils import run_bass_kernel_spmd

F32 = mybir.dt.float32
F32R = mybir.dt.float32r
BF16 = mybir.dt.bfloat16
NPBF16 = ml_dtypes.bfloat16
EXP = mybir.ActivationFunctionType.Exp
ACT_COPY = mybir.ActivationFunctionType.Copy

DIM = 2048
N_HEADS = 16
HEAD_DIM = 128
BATCH = 2
SEQ = 2048
G_HEADS = 4          # heads per core
GM = G_HEADS * HEAD_DIM  # 512 output cols per core
DC = DIM // 128      # 16 contraction chunks
SC512 = SEQ // 512   # 4
SC128 = SEQ // 128   # 16
NJ2 = SC128 // 2     # 8 j-chunk pairs
INV_SQRT_HD = float(1.0 / np.sqrt(HEAD_DIM))

# even<->odd partition swap within each 32-partition group
_SWAP_MASK = [i ^ 1 for i in range(32)]


def build_fast():
    nc = bacc.Bacc("TRN2", target_bir_lowering=False, debug=False)

    xt_d = nc.dram_tensor("xt", [DC, 128, SEQ], BF16, kind="ExternalInput").ap()
    wq_d = nc.dram_tensor("wq", [G_HEADS, 128, DC * 128], BF16, kind="ExternalInput").ap()
    wk_d = nc.dram_tensor("wk", [G_HEADS, 128, DC * 128], BF16, kind="ExternalInput").ap()
    wv_d = nc.dram_tensor("wv", [128, DC * GM], BF16, kind="ExternalInput").ap()
    wo_d = nc.dram_tensor("wo", [G_HEADS, 128, SEQ], BF16, kind="ExternalInput").ap()
    ce_d = nc.dram_tensor("ce", [128, SEQ], BF16, kind="ExternalInput").ap()
    s2_d = nc.dram_tensor("s2", [128, SEQ], BF16, kind="ExternalInput").ap()
    ones_d = nc.dram_tensor("ones", [128, 1], BF16, kind="ExternalInput").ap()
    out_d = nc.dram_tensor("out", [SC128, 128, SEQ], BF16, kind="ExternalOutput").ap()

    with tile.TileContext(nc) as tc:
        consts = tc.alloc_tile_pool(name="consts", bufs=1)
        persist = tc.alloc_tile_pool(name="persist", bufs=1)
        rope_t = tc.alloc_tile_pool(name="ropet", bufs=2)

        q_t = [persist.tile([128, SEQ], BF16, tag=f"q{h}", name=f"q{h}") for h in range(G_HEADS)]
        k_t = [persist.tile([128, SEQ], BF16, tag=f"k{h}", name=f"k{h}") for h in range(G_HEADS)]
        v_t = [persist.tile([128, GM], BF16, tag=f"v{s}", name=f"v{s}") for s in range(SC128)]
        ce_t = persist.tile([128, SEQ], BF16, tag="ce", name="ce")
        s2_t = persist.tile([128, SEQ], BF16, tag="s2", name="s2")

        xf_pool = tc.alloc_tile_pool(name="xf", bufs=1)
        wqk_pool = tc.alloc_tile_pool(name="wqk", bufs=3)
        wv_pool = tc.alloc_tile_pool(name="wvp", bufs=1)
        ps_proj = tc.alloc_tile_pool(name="psproj", bufs=8, space="PSUM")

        # ---------- startup DMAs ----------
        xf = [xf_pool.tile([128, SEQ], BF16, tag=f"x{dc}", name=f"x{dc}") for dc in range(DC)]
        w0q = wqk_pool.tile([128, DC * 128], BF16, tag="w", name="w0q")
        w0k = wqk_pool.tile([128, DC * 128], BF16, tag="w", name="w0k")
        ones_t = consts.tile([128, 1], BF16, tag="ones", name="ones")
        junk_t = consts.tile([128, 512], BF16, tag="junk", name="junk")
        with tc.high_priority():
            nc.sync.dma_start(xf[0][:, 0:1024], xt_d[0][:, 0:1024])
            nc.sync.dma_start(w0q[:, 0:512], wq_d[0][:, 0:512])
            nc.scalar.dma_start(xf[0][:, 1024:], xt_d[0][:, 1024:])
            nc.scalar.dma_start(w0k[:, 0:512], wk_d[0][:, 0:512])
            nc.gpsimd.dma_start(ones_t[:], ones_d)
            # PE p-state warm-up: a few throwaway matmuls so the ramp epoch
            # starts at ~1.5us and the first real matmuls run at full rate
            nc.vector.memset(junk_t[:], 0.5)
            warm_ps = ps_proj.tile([128, 512], F32, tag="pp", name="warm_ps")
            for i in range(5):
                nc.tensor.matmul(warm_ps[:], junk_t[:, 0:128], junk_t[:],
                                 start=(i == 0), stop=(i == 4))
        warm_t = consts.tile([128, 1], F32, tag="warm", name="warm")
        nc.scalar.activation(out=warm_t[:], in_=ones_t[:], func=EXP)
        for dc in range(1, 4):
            eng = nc.scalar if dc % 2 == 1 else nc.sync
            eng.dma_start(xf[dc][:], xt_d[dc])
        nc.sync.dma_start(w0q[:, 512:], wq_d[0][:, 512:])
        nc.scalar.dma_start(w0k[:, 512:], wk_d[0][:, 512:])
        for dc in range(4, DC):
            eng = nc.scalar if dc % 2 == 1 else nc.sync
            eng.dma_start(xf[dc][:], xt_d[dc])
        # h1 weights then rope tables — all needed right at the end of G0.
        # w1k reuses w0q's pool slot (frees at G0 end) so it must not sit
        # ahead of ce/s2 in a queue.
        w1q = wqk_pool.tile([128, DC * 128], BF16, tag="w", name="w1q")
        nc.sync.dma_start(w1q[:], wq_d[1])
        w1k = wqk_pool.tile([128, DC * 128], BF16, tag="w", name="w1k")
        nc.sync.dma_start(w1k[:], wk_d[1])
        nc.scalar.dma_start(ce_t[:], ce_d)
        nc.scalar.dma_start(s2_t[:], s2_d)

        def rope_split(ps, dst, sc, t3_pool=False):
            """ACT-copy the PSUM chain result into its bf16 slice, start the
            sin-product on GpSimd; return a closure emitting the remaining
            DVE ops (all-SBUF bf16, so they run in the 2x mode). GPSIMD must
            never touch PSUM (BIR verifier rule)."""
            sl = slice(sc * 512, (sc + 1) * 512)
            nc.scalar.activation(out=dst[:, sl], in_=ps[:], func=ACT_COPY)
            t1 = rope_t.tile([128, 512], BF16, tag="t1", name="t1")
            nc.gpsimd.tensor_mul(out=t1[:], in0=dst[:, sl], in1=s2_t[:, sl])

            def back():
                t3 = rope_t.tile([128, 512], BF16, tag="t3", name="t3")
                nc.vector.tensor_mul(out=t3[:], in0=dst[:, sl], in1=ce_t[:, sl])
                t2 = rope_t.tile([128, 512], BF16, tag="t2", name="t2")
                nc.vector.stream_shuffle(t2[:], t1[:], _SWAP_MASK)
                nc.vector.tensor_add(out=dst[:, sl], in0=t3[:], in1=t2[:])
            return back

        def rope(ps, dst, sc, t3_pool=False):
            rope_split(ps, dst, sc, t3_pool)()

        # ---------- G0: h0 q,k — dc-major, paced by x arrival ----------
        chains0 = [("q", sc) for sc in range(SC512)] + [("k", sc) for sc in range(SC512)]
        ps0 = [ps_proj.tile([128, 512], F32, tag="pp", name=f"pp{ci}")
               for ci in range(len(chains0))]
        for dc in range(DC):
            for ci, (qk, sc) in enumerate(chains0):
                w = w0q if qk == "q" else w0k
                nc.tensor.matmul(
                    ps0[ci][:], w[:, dc * 128:(dc + 1) * 128],
                    xf[dc][:, sc * 512:(sc + 1) * 512],
                    start=(dc == 0), stop=(dc == DC - 1),
                )
        for ci, (qk, sc) in enumerate(chains0):
            rope(ps0[ci], q_t[0] if qk == "q" else k_t[0], sc)

        # ---------- G1: h1 q,k — chain-major ----------
        for qk, w, dst in (("q", w1q, q_t[1]), ("k", w1k, k_t[1])):
            for sc in range(SC512):
                ps = ps_proj.tile([128, 512], F32, tag="pp", name="pp")
                for dc in range(DC):
                    nc.tensor.matmul(
                        ps[:], w[:, dc * 128:(dc + 1) * 128],
                        xf[dc][:, sc * 512:(sc + 1) * 512],
                        start=(dc == 0), stop=(dc == DC - 1),
                    )
                rope(ps, dst, sc)

        # ---------- V: 16 chains, ACT copies to v tiles ----------
        wvt = wv_pool.tile([128, DC * GM], BF16, tag="wv", name="wvt")
        for i in range(4):
            nc.scalar.dma_start(
                wvt[:, i * 2048:(i + 1) * 2048], wv_d[:, i * 2048:(i + 1) * 2048]
            )
        for s in range(SC128):
            ps = ps_proj.tile([128, GM], F32, tag="pp", name="pp")
            for dc in range(DC):
                nc.tensor.matmul(
                    ps[:], xf[dc][:, s * 128:(s + 1) * 128],
                    wvt[:, dc * GM:(dc + 1) * GM],
                    start=(dc == 0), stop=(dc == DC - 1),
                )
            nc.scalar.activation(out=v_t[s][:], in_=ps[:], func=ACT_COPY)
        wv_pool.release()

        # weights for the h2/h3 fill chains (k first: needed by hp1 earliest)
        w2k = wqk_pool.tile([128, DC * 128], BF16, tag="w", name="w2k")
        nc.sync.dma_start(w2k[:], wk_d[2])
        w3k = wqk_pool.tile([128, DC * 128], BF16, tag="w", name="w3k")
        nc.scalar.dma_start(w3k[:], wk_d[3])
        w2q = wqk_pool.tile([128, DC * 128], BF16, tag="w", name="w2q")
        nc.sync.dma_start(w2q[:], wq_d[2])
        w3q = wqk_pool.tile([128, DC * 128], BF16, tag="w", name="w3q")
        nc.scalar.dma_start(w3q[:], wq_d[3])

        ps_proj.release()

        # ---------- attention pools ----------
        o_pool = tc.alloc_tile_pool(name="oT", bufs=1)
        o_t = [o_pool.tile([128, SEQ], BF16, tag=f"o{h}", name=f"o{h}") for h in range(G_HEADS)]
        e_pool = tc.alloc_tile_pool(name="e", bufs=4)
        acc_pool = tc.alloc_tile_pool(name="acce", bufs=3)
        eh_pool = tc.alloc_tile_pool(name="eh", bufs=2)
        red_pool = tc.alloc_tile_pool(name="red", bufs=3)
        f_pool = tc.alloc_tile_pool(name="fp", bufs=4)
        ps_st = tc.alloc_tile_pool(name="ps_st", bufs=2, space="PSUM")
        ps_av = tc.alloc_tile_pool(name="ps_av", bufs=2, space="PSUM")
        ps_fill = tc.alloc_tile_pool(name="ps_fill", bufs=2, space="PSUM")

        wo_pool = tc.alloc_tile_pool(name="wo", bufs=1)
        wo_t = []  # tiles created after xf releases (SBUF headroom)

        def p3_mm(s, nck, psum_pool):
            ps = psum_pool.tile([128, 512], F32, tag="fill", name="psf")
            for mc in range(G_HEADS):
                nc.tensor.matmul(
                    ps[:], o_t[mc][:, s * 128:(s + 1) * 128],
                    wo_t[mc][:, nck * 512:(nck + 1) * 512],
                    start=(mc == 0), stop=(mc == G_HEADS - 1),
                )
            return ps

        def p3_out(s, nck, ps, eng, fstate, paired=True):
            # pairs of 512-col results share one [128,1024] staging tile and
            # one store, halving HWDGE issue pressure; the last tail chains
            # store unpaired so the end-of-kernel DMA drain is shorter
            if not paired:
                fo = f_pool.tile([128, 512], BF16, tag="fs", name="fs")
                if eng == 0:
                    nc.scalar.activation(out=fo[:], in_=ps[:], func=ACT_COPY)
                else:
                    nc.vector.tensor_copy(out=fo[:], in_=ps[:])
                st_eng = nc.scalar if eng else nc.sync
                st_eng.dma_start(out_d[s][:, nck * 512:(nck + 1) * 512], fo[:])
                return
            if nck % 2 == 0:
                fstate["f"] = f_pool.tile([128, 1024], BF16, tag="f", name="f")
            fo = fstate["f"]
            half = (nck % 2) * 512
            dst = fo[:, half:half + 512]
            if eng == 0:
                nc.scalar.activation(out=dst, in_=ps[:], func=ACT_COPY)
            else:
                nc.vector.tensor_copy(out=dst, in_=ps[:])
            if nck % 2 == 1:
                st_eng = nc.scalar if (nck // 2) % 2 == 0 else nc.sync
                st_eng.dma_start(out_d[s][:, (nck - 1) * 512:(nck + 1) * 512], fo[:])

        def make_g23_fill(chains):
            # h2/h3 projection chains, 4 matmul-groups each. The rope-front
            # consumers of chain i-1 are emitted after chain i's first group
            # so the two ps_fill banks cycle without stalling PE.
            state = {"i": 0, "g": 0, "ps": None,
                     "front": None, "back": None, "done": False}

            def flush_front():
                if state["front"] is not None:
                    state["back"] = state["front"]()
                    state["front"] = None

            def flush_back():
                if state["back"] is not None:
                    state["back"]()
                    state["back"] = None

            def fill():
                i = state["i"]
                if i >= len(chains):
                    if state["front"] or state["back"]:
                        flush_front()
                        flush_back()
                        return True
                    return False
                h, qk, sc, w = chains[i]
                g = state["g"]
                if g == 0:
                    state["ps"] = ps_fill.tile([128, 512], F32, tag="fill", name="psf")
                ps = state["ps"]
                for dc in range(4 * g, 4 * g + 4):
                    nc.tensor.matmul(
                        ps[:], w[:, dc * 128:(dc + 1) * 128],
                        xf[dc][:, sc * 512:(sc + 1) * 512],
                        start=(dc == 0), stop=(dc == DC - 1),
                    )
                if g == 0:
                    flush_front()
                elif g == 1:
                    flush_back()
                elif g == 3:
                    dst = q_t[h] if qk == "q" else k_t[h]
                    state["front"] = (
                        lambda p=ps, d=dst, s_=sc: rope_split(p, d, s_, t3_pool=True)
                    )
                    state["i"] += 1
                state["g"] = (g + 1) % 4
                return True
            return fill

        def make_p3_fill(ic, skip=4, psum_pool=None, engs=(0, 1, 0, 1),
                         unpair_from=16):
            # chain n's copy+store lag one slot behind its matmuls
            pool = psum_pool if psum_pool is not None else ps_fill
            state = {"n": -skip, "prev": None, "f": None}

            def out_prev():
                s, nck, ps, k = state["prev"]
                p3_out(s, nck, ps, engs[k % len(engs)], state,
                       paired=(k < unpair_from))
                state["prev"] = None

            def fill():
                n = state["n"]
                if n >= 16:
                    if state["prev"] is not None:
                        out_prev()
                        return True
                    return False
                state["n"] += 1
                if n < 0:
                    return True
                s, nck = ic * 4 + n // 4, n % 4
                ps = p3_mm(s, nck, pool)
                if state["prev"] is not None:
                    out_prev()
                state["prev"] = (s, nck, ps, n)
                return True
            return fill

        def noop_fill():
            return False

        def emit_block(ha, hb, ic, fill, prev_tail):
            """One attention block for local heads (ha, hb) on query chunk ic.
            prev_tail: (eh_fns, den_fns, fin_fns) per-head closures of the
            previous block's denominator/normalize, woven in here so the PE
            never waits on them at a block seam. Returns this block's tail."""
            isl = slice(ic * 512, (ic + 1) * 512)
            acc = {}
            acc_e = {}
            e_of = {}
            for h in (ha, hb):
                acc[h] = ps_av.tile([128, 512], F32, tag="acc", name="acc")

            def emit_st(h, jc2, copy_pool=False):
                ja, jb = 2 * jc2, 2 * jc2 + 1
                st = ps_st.tile([128, 1024], F32, tag="st", name="st")
                nc.tensor.matmul(
                    st[:, 0:512], k_t[h][:, ja * 128:(ja + 1) * 128], q_t[h][:, isl],
                    start=True, stop=True,
                )
                nc.tensor.matmul(
                    st[:, 512:1024], k_t[h][:, jb * 128:(jb + 1) * 128], q_t[h][:, isl],
                    start=True, stop=True,
                )
                e = e_pool.tile([128, 1024], BF16, tag="e", name="e")
                nc.scalar.activation(out=e[:], in_=st[:], func=EXP)
                if jc2 == 0:
                    a = acc_pool.tile([128, 1024], BF16, tag="acce", name="acce")
                    acc_e[h] = a
                    # first copy off the DVE queue head so the previous
                    # block's eh/recip chain runs unblocked
                    eng = nc.gpsimd if copy_pool else nc.vector
                    eng.tensor_copy(out=a[:], in_=e[:])
                else:
                    nc.vector.tensor_add(out=acc_e[h][:], in0=acc_e[h][:], in1=e[:])
                e_of[(h, jc2)] = e

            def emit_av(h, jc2):
                ja, jb = 2 * jc2, 2 * jc2 + 1
                e = e_of.pop((h, jc2))
                nc.tensor.matmul(
                    acc[h][:], v_t[ja][:, h * 128:(h + 1) * 128], e[:, 0:512],
                    start=(jc2 == 0), stop=False,
                )
                nc.tensor.matmul(
                    acc[h][:], v_t[jb][:, h * 128:(h + 1) * 128], e[:, 512:1024],
                    start=False, stop=(jc2 == NJ2 - 1),
                )

            p_eh, p_den, p_fin = prev_tail if prev_tail else (None, None, None)
            if p_eh:
                p_eh[0]()
            emit_st(ha, 0, copy_pool=True)
            if p_eh:
                p_eh[1]()
            fill()
            if p_den:
                p_den[0]()
            emit_st(hb, 0)
            fill()
            if p_den:
                p_den[1]()
            if p_fin:
                p_fin[0]()
                p_fin[1]()
            for jc2 in range(1, NJ2):
                emit_st(ha, jc2)
                fill()
                emit_av(ha, jc2 - 1)
                emit_st(hb, jc2)
                fill()
                emit_av(hb, jc2 - 1)
            emit_av(ha, NJ2 - 1)
            fill()
            emit_av(hb, NJ2 - 1)
            fill()

            # build this block's tail closures
            eh_of = {}
            red_of = {}

            def mk_eh(h):
                def f():
                    eh = eh_pool.tile([128, 512], BF16, tag="eh", name="eh")
                    nc.vector.tensor_add(
                        out=eh[:], in0=acc_e[h][:, 0:512], in1=acc_e[h][:, 512:1024]
                    )
                    eh_of[h] = eh
                return f

            def mk_red(h):
                def f():
                    # fused partition-reduce + broadcast of the softmax
                    # denominator on the otherwise-idle GpSimd engine
                    red = red_pool.tile([128, 512], F32, tag="red", name="red")
                    nc.gpsimd.partition_all_reduce(
                        red[:], eh_of[h][:], 128, bass_isa.ReduceOp.add
                    )
                    nc.vector.reciprocal_approx_fast(out=red[:], in_=red[:])
                    red_of[h] = red
                return f

            def mk_fin(h):
                def f():
                    nc.vector.tensor_mul(
                        out=o_t[h][:, isl], in0=acc[h][:], in1=red_of[h][:]
                    )
                return f

            return ([mk_eh(ha), mk_eh(hb)],
                    [mk_red(ha), mk_red(hb)],
                    [mk_fin(ha), mk_fin(hb)])

        def run_tail(t):
            eh_fns, den_fns, fin_fns = t
            for f in eh_fns:
                f()
            for f in den_fns:
                f()
            for f in fin_fns:
                f()

        # hp0: heads 0,1 with h2/h3 projection fill; the last two chains
        # (q2/q3 sc3, not needed until hp1-ic3) spill into hp1-ic0 so that
        # block has PE filler too
        g23_chains = []
        for qk, wmap in (("k", {2: w2k, 3: w3k}), ("q", {2: w2q, 3: w3q})):
            for h in (2, 3):
                for sc in range(SC512):
                    g23_chains.append((h, qk, sc, wmap[h]))
        # only q2/q3 sc2-3 may spill into hp1-ic0: hp1-ic0/ic1 read q2/q3
        # sc0-1, which must be written before hp1 starts
        _spill = (10, 11, 14, 15)
        g23 = make_g23_fill([c for i, c in enumerate(g23_chains)
                             if i not in _spill])
        g23b = make_g23_fill([g23_chains[i] for i in _spill])
        tail = None
        for ic in range(SC512):
            tail = emit_block(0, 1, ic, g23, tail)
        while g23():
            pass

        for mc in range(G_HEADS):
            w = wo_pool.tile([128, SEQ], BF16, tag=f"wo{mc}", name=f"wo{mc}")
            eng = nc.sync if mc % 2 == 0 else nc.scalar
            eng.dma_start(w[:], wo_d[mc])
            wo_t.append(w)

        # hp1: heads 2,3 with output-projection fill (one ic behind)
        fills = [g23b] + [make_p3_fill(ic) for ic in range(SC512 - 1)]
        for ic in range(SC512):
            tail = emit_block(2, 3, ic, fills[ic], tail)
            if ic < SC512 - 1:
                while fills[ic]():
                    pass

        # leftover p3(ic2) chains cover the final tail's eh->den latency
        left = fills[SC512 - 1]
        left()
        run_tail(tail)
        while left():
            pass

        ps_fill.release()
        ps_av.release()
        ps_st.release()

        ps3 = tc.alloc_tile_pool(name="ps3", bufs=4, space="PSUM")
        tail_fill = make_p3_fill(SC512 - 1, skip=0, psum_pool=ps3,
                                 engs=(0, 1, 0, 1), unpair_from=12)
        while tail_fill():
            pass
        ps3.release()

        for p in (wo_pool, f_pool, red_pool, eh_pool, acc_pool,
                  e_pool, o_pool, wqk_pool, xf_pool, rope_t, persist, consts):
            p.release()

    nc.compile()
    return nc


def build_masked():
    """Previous two-pass fp32r kernel — only used when mask is nonzero."""
    nc = bacc.Bacc("TRN2", target_bir_lowering=False, debug=False)

    xt_d = nc.dram_tensor("xt", [DC, 128, SEQ], F32R, kind="ExternalInput").ap()
    wq_d = nc.dram_tensor("wq", [DC, G_HEADS, 128, 128], F32R, kind="ExternalInput").ap()
    wk_d = nc.dram_tensor("wk", [DC, G_HEADS, 128, 128], F32R, kind="ExternalInput").ap()
    wv_d = nc.dram_tensor("wv", [DC, 128, GM], F32R, kind="ExternalInput").ap()
    wo_d = nc.dram_tensor("wo", [G_HEADS, 128, SEQ], F32R, kind="ExternalInput").ap()
    ce_d = nc.dram_tensor("ce", [128, SEQ], F32R, kind="ExternalInput").ap()
    s2_d = nc.dram_tensor("s2", [128, SEQ], F32R, kind="ExternalInput").ap()
    ones_d = nc.dram_tensor("ones", [128, 1], F32R, kind="ExternalInput").ap()
    mt_d = nc.dram_tensor("mt", [SC128, 128, SEQ], F32, kind="ExternalInput").ap()
    out_d = nc.dram_tensor("out", [SC128, 128, SEQ], F32, kind="ExternalOutput").ap()

    with tile.TileContext(nc) as tc:
        with (
            tc.tile_pool(name="persist", bufs=1) as persist,
            tc.tile_pool(name="consts", bufs=1) as consts,
        ):
            ones_t = consts.tile([128, 1], F32R, tag="ones")
            nc.sync.dma_start(ones_t[:], ones_d)
            warm_t = consts.tile([128, 1], F32, tag="warm")
            nc.scalar.activation(
                out=warm_t[:], in_=ones_t[:],
                func=EXP,
            )

            q_t = [persist.tile([128, SEQ], F32R, tag=f"q{h}", name=f"q{h}") for h in range(G_HEADS)]
            k_t = [persist.tile([128, SEQ], F32R, tag=f"k{h}", name=f"k{h}") for h in range(G_HEADS)]
            v_t = [persist.tile([128, GM], F32R, tag=f"v{s}", name=f"v{s}") for s in range(SC128)]

            with (
                tc.tile_pool(name="rope_c", bufs=1) as rope_c,
                tc.tile_pool(name="xt", bufs=8) as xt_pool,
                tc.tile_pool(name="wqk", bufs=2) as wqk_pool,
                tc.tile_pool(name="wv", bufs=1) as wv_pool,
                tc.tile_pool(name="ps1", bufs=4, space="PSUM") as ps1,
                tc.tile_pool(name="rope_t", bufs=1) as rope_t,
            ):
                ce_t = rope_c.tile([128, SEQ], F32R, tag="ce")
                s2_t = rope_c.tile([128, SEQ], F32R, tag="s2")

                def rope(t, sl):
                    t1 = rope_t.tile([128, 512], F32, tag="t1", name="t1")
                    nc.gpsimd.tensor_mul(out=t1[:], in0=t[:, sl], in1=s2_t[:, sl])
                    t2 = rope_t.tile([128, 512], F32, tag="t2", name="t2")
                    nc.vector.stream_shuffle(t2[:], t1[:], _SWAP_MASK)
                    t3 = rope_t.tile([128, 512], F32, tag="t3", name="t3")
                    nc.vector.tensor_mul(out=t3[:], in0=t[:, sl], in1=ce_t[:, sl])
                    nc.vector.tensor_add(out=t[:, sl], in0=t3[:], in1=t2[:])

                for half in range(2):
                    dcs = list(range(half * 8, half * 8 + 8))
                    prio = tc.high_priority() if half == 0 else None
                    if prio is not None:
                        prio.__enter__()
                    wt_first = wqk_pool.tile([128, 8, 128], F32R, tag="w", name="wt")
                    nc.sync.dma_start(
                        wt_first[:],
                        wq_d[dcs[0] : dcs[0] + 8, 0].rearrange("c p m -> p c m"),
                    )
                    xtsA, xtsB = [], []
                    for qi, dc in enumerate(dcs):
                        xa = xt_pool.tile([128, 1024], F32R, tag="xa", name="xa")
                        eng = nc.sync if qi % 2 == 0 else nc.scalar
                        eng.dma_start(xa[:], xt_d[dc][:, 0:1024])
                        xtsA.append(xa)
                    if prio is not None:
                        prio.__exit__(None, None, None)
                    for qi, dc in enumerate(dcs):
                        xb = xt_pool.tile([128, 1024], F32R, tag="xb", name="xb")
                        eng = nc.sync if qi % 2 == 1 else nc.scalar
                        eng.dma_start(xb[:], xt_d[dc][:, 1024:2048])
                        xtsB.append(xb)

                    def xslice(i, sl_start, width):
                        if sl_start < 1024:
                            return xtsA[i][:, sl_start : sl_start + width]
                        return xtsB[i][:, sl_start - 1024 : sl_start - 1024 + width]
                    wvt = wv_pool.tile([128, 8, GM], F32R, tag="wv", name="wvt")
                    nc.scalar.dma_start(
                        wvt[:], wv_d[dcs[0] : dcs[0] + 8].rearrange("c p m -> p c m")
                    )
                    v_next = 0

                    def emit_v(n):
                        nonlocal v_next
                        for s in range(v_next, v_next + n):
                            ps = ps1.tile([128, GM], F32, tag="ps", name="ps")
                            for i in range(8):
                                nc.tensor.matmul(
                                    ps[:], xslice(i, s * 128, 128), wvt[:, i, :],
                                    start=(i == 0), stop=(i == 7),
                                )
                            if half == 0:
                                nc.vector.tensor_copy(out=v_t[s][:], in_=ps[:])
                            else:
                                nc.vector.tensor_add(
                                    out=v_t[s][:], in0=ps[:], in1=v_t[s][:]
                                )
                        v_next += n

                    for h in range(G_HEADS):
                        for wi, (wd, dst) in enumerate(((wq_d, q_t[h]), (wk_d, k_t[h]))):
                            if h == 0 and wi == 0:
                                wt = wt_first
                            else:
                                wt = wqk_pool.tile([128, 8, 128], F32R, tag="w", name="wt")
                                nc.sync.dma_start(
                                    wt[:],
                                    wd[dcs[0] : dcs[0] + 8, h].rearrange("c p m -> p c m"),
                                )
                            for sc in range(SC512):
                                ps = ps1.tile([128, 512], F32, tag="ps", name="ps")
                                sl = bass.ts(sc, 512)
                                for i in range(8):
                                    nc.tensor.matmul(
                                        ps[:], wt[:, i, :], xslice(i, sc * 512, 512),
                                        start=(i == 0), stop=(i == 7),
                                    )
                                if half == 0:
                                    nc.vector.tensor_copy(out=dst[:, sl], in_=ps[:])
                                else:
                                    nc.vector.tensor_add(
                                        out=dst[:, sl], in0=ps[:], in1=dst[:, sl]
                                    )
                                    rope(dst, sl)
                        if (half == 0 and h >= 2) or half == 1:
                            emit_v({0: 8, 1: 4}[half])
                        if half == 0 and h == G_HEADS - 1:
                            nc.scalar.dma_start(ce_t[:], ce_d)
                            nc.scalar.dma_start(s2_t[:], s2_d)

            o_pool = tc.alloc_tile_pool(name="oT", bufs=1)
            o_t = [o_pool.tile([128, SEQ], F32R, tag=f"o{h}", name=f"o{h}") for h in range(G_HEADS)]
            wo_pool = tc.alloc_tile_pool(name="wo", bufs=1)
            wo_t = []
            for mc in range(G_HEADS):
                w = wo_pool.tile([128, SEQ], F32R, tag=f"wo{mc}", name=f"wo{mc}")
                nc.sync.dma_start(w[:], wo_d[mc])
                wo_t.append(w)
            with (
                tc.tile_pool(name="est", bufs=5) as est_pool,
                tc.tile_pool(name="nrm", bufs=3) as nrm_pool,
                tc.tile_pool(name="ps_st", bufs=2, space="PSUM") as ps_st,
                tc.tile_pool(name="ps_av", bufs=2, space="PSUM") as ps_av,
                tc.tile_pool(name="ps_dn", bufs=2, space="PSUM") as ps_dn,
            ):
                mask_pool = tc.alloc_tile_pool(name="mask", bufs=2)

                for ic in range(SC512):
                    isl = bass.ts(ic, 512)
                    for hp in range(G_HEADS // 2):
                        heads = (2 * hp, 2 * hp + 1)
                        acc = {}
                        den = {}
                        e_of = {}
                        m_of = {}
                        for h in heads:
                            acc[h] = ps_av.tile([128, 512], F32, tag="acc", name="acc")
                            den[h] = ps_dn.tile([1, 512], F32, tag="den", name="den")

                        def emit_st(h, jc2):
                            ja, jb = 2 * jc2, 2 * jc2 + 1
                            st = ps_st.tile([128, 1024], F32, tag="st", name="st")
                            nc.tensor.matmul(
                                st[:, 0:512],
                                k_t[h][:, bass.ts(ja, 128)], q_t[h][:, isl],
                                start=True, stop=True,
                            )
                            nc.tensor.matmul(
                                st[:, 512:1024],
                                k_t[h][:, bass.ts(jb, 128)], q_t[h][:, isl],
                                start=True, stop=True,
                            )
                            e = est_pool.tile([128, 1024], F32R, tag="e", name="e")
                            if jc2 not in m_of:
                                mtl = mask_pool.tile(
                                    [128, 1024], F32, tag="m", name="mtl"
                                )
                                nc.sync.dma_start(mtl[:, 0:512], mt_d[ja, :, isl])
                                nc.sync.dma_start(mtl[:, 512:1024], mt_d[jb, :, isl])
                                m_of[jc2] = mtl
                            nc.vector.tensor_add(
                                out=e[:], in0=st[:], in1=m_of[jc2][:]
                            )
                            nc.scalar.activation(
                                out=e[:], in_=e[:],
                                func=EXP,
                            )
                            eh = est_pool.tile([128, 512], F32R, tag="eh", name="eh", bufs=3)
                            nc.vector.tensor_add(
                                out=eh[:], in0=e[:, 0:512], in1=e[:, 512:1024]
                            )
                            e_of[h] = (e, eh)

                        def emit_denav(h, jc2):
                            ja, jb = 2 * jc2, 2 * jc2 + 1
                            e, eh = e_of[h]
                            last = jc2 == SC128 // 2 - 1
                            nc.tensor.matmul(
                                den[h][:], ones_t[:], eh[:],
                                start=(jc2 == 0), stop=last,
                            )
                            nc.tensor.matmul(
                                acc[h][:], v_t[ja][:, bass.ts(h, 128)], e[:, 0:512],
                                start=(jc2 == 0), stop=False,
                            )
                            nc.tensor.matmul(
                                acc[h][:], v_t[jb][:, bass.ts(h, 128)], e[:, 512:1024],
                                start=False, stop=last,
                            )

                        h0, h1 = heads
                        emit_st(h0, 0)
                        for jc2 in range(NJ2):
                            if jc2 > 0:
                                emit_st(h0, jc2)
                                emit_denav(h1, jc2 - 1)
                            emit_st(h1, jc2)
                            emit_denav(h0, jc2)
                        emit_denav(h1, NJ2 - 1)

                        for h in heads:
                            rec = nrm_pool.tile([1, 512], F32, tag="rec", name="rec")
                            nc.vector.reciprocal_approx_fast(out=rec[:], in_=den[h][:])
                            bc = nrm_pool.tile([128, 512], F32, tag="bc", name="bc")
                            nc.gpsimd.partition_broadcast(bc[:], rec[:])
                            nc.vector.tensor_mul(
                                out=o_t[h][:, isl], in0=acc[h][:], in1=bc[:]
                            )
                mask_pool.release()

            with (
                tc.tile_pool(name="fin", bufs=10) as fin_pool,
                tc.tile_pool(name="ps3", bufs=8, space="PSUM") as ps3,
            ):
                for s in range(SC128):
                    ssl = bass.ts(s, 128)
                    for nck in range(SC512):
                        nsl = bass.ts(nck, 512)
                        ps = ps3.tile([128, 512], F32, tag="ps3", name="ps3")
                        for mc in range(G_HEADS):
                            nc.tensor.matmul(
                                ps[:], o_t[mc][:, ssl], wo_t[mc][:, nsl],
                                start=(mc == 0), stop=(mc == G_HEADS - 1),
                            )
                        f = fin_pool.tile([128, 512], F32, tag="f", name="f")
                        nc.vector.tensor_copy(out=f[:], in_=ps[:])
                        nc.sync.dma_start(out_d[s, :, nsl], f[:])
            wo_pool.release()
            o_pool.release()

    nc.compile()
    return nc


_CACHE = {}


def _get_nc(with_mask: bool):
    if with_mask not in _CACHE:
        _CACHE[with_mask] = build_masked() if with_mask else build_fast()
    return _CACHE[with_mask]


def kernel(in_token, freqs_cos, freqs_sin, mask, wq, wk, wv, wo):
    return _run(in_token, freqs_cos, freqs_sin, mask, wq, wk, wv, wo)


def run_traced(in_token, freqs_cos, freqs_sin, mask, wq, wk, wv, wo):
    """Test-only: run with NTFF tracing, return (output, BassKernelResults)."""
    return _run(in_token, freqs_cos, freqs_sin, mask, wq, wk, wv, wo, trace=True)


def _run(in_token, freqs_cos, freqs_sin, mask, wq, wk, wv, wo, trace=False):
    in_token = np.ascontiguousarray(np.asarray(in_token, dtype=np.float32))
    freqs_cos = np.asarray(freqs_cos, dtype=np.float32)
    freqs_sin = np.asarray(freqs_sin, dtype=np.float32)
    mask = np.asarray(mask, dtype=np.float32)
    wq = np.asarray(wq, dtype=np.float32)
    wk = np.asarray(wk, dtype=np.float32)
    wv = np.asarray(wv, dtype=np.float32)
    wo = np.asarray(wo, dtype=np.float32)

    with_mask = bool(np.any(mask))
    nc = _get_nc(with_mask)

    if with_mask:
        return _run_masked(nc, in_token, freqs_cos, freqs_sin, mask,
                           wq, wk, wv, wo, trace)

    ce = np.repeat(freqs_cos.T, 2, axis=0).astype(NPBF16)  # (128, S)
    s2 = np.empty((HEAD_DIM, SEQ), np.float32)
    s2[0::2] = freqs_sin.T
    s2[1::2] = -freqs_sin.T
    s2 = s2.astype(NPBF16)
    ones = np.ones((128, 1), NPBF16)

    in_maps = []
    xts = [
        np.ascontiguousarray(in_token[b].T).reshape(DC, 128, SEQ).astype(NPBF16)
        for b in range(BATCH)
    ]
    for b in range(BATCH):
        for g in range(G_HEADS):
            rows = slice(g * GM, (g + 1) * GM)
            wqt = np.ascontiguousarray(
                (wq[rows] * INV_SQRT_HD).T.reshape(DC, 128, G_HEADS, 128)
                .transpose(2, 1, 0, 3).reshape(G_HEADS, 128, DC * 128)
            ).astype(NPBF16)
            wkt = np.ascontiguousarray(
                wk[rows].T.reshape(DC, 128, G_HEADS, 128)
                .transpose(2, 1, 0, 3).reshape(G_HEADS, 128, DC * 128)
            ).astype(NPBF16)
            wvt = np.ascontiguousarray(
                wv[rows].T.reshape(DC, 128, GM).transpose(1, 0, 2)
                .reshape(128, DC * GM)
            ).astype(NPBF16)
            wot = np.ascontiguousarray(wo[:, rows].T).reshape(
                G_HEADS, 128, SEQ
            ).astype(NPBF16)
            m = {
                "xt": xts[b], "wq": wqt, "wk": wkt, "wv": wvt, "wo": wot,
                "ce": ce, "s2": s2, "ones": ones,
            }
            in_maps.append(m)

    res = run_bass_kernel_spmd(nc, in_maps, core_ids=list(range(8)), trace=trace)

    out = np.zeros((BATCH, SEQ, DIM), np.float32)
    for b in range(BATCH):
        acc = None
        for g in range(G_HEADS):
            p = res.results[b * G_HEADS + g]["out"].astype(np.float32).reshape(SEQ, DIM)
            acc = p if acc is None else acc + p
        out[b] = acc
    if trace:
        return out, res
    return out


def _run_masked(nc, in_token, freqs_cos, freqs_sin, mask, wq, wk, wv, wo, trace):
    ce = np.repeat(freqs_cos.T, 2, axis=0).astype(np.float32)  # (128, S)
    s2 = np.empty((HEAD_DIM, SEQ), np.float32)
    s2[0::2] = freqs_sin.T
    s2[1::2] = -freqs_sin.T
    ones = np.ones((128, 1), np.float32)
    mt = np.ascontiguousarray(mask.T).reshape(SC128, 128, SEQ)

    in_maps = []
    xts = [
        np.ascontiguousarray(in_token[b].T).reshape(DC, 128, SEQ)
        for b in range(BATCH)
    ]
    for b in range(BATCH):
        for g in range(G_HEADS):
            rows = slice(g * GM, (g + 1) * GM)
            wqt = np.ascontiguousarray(
                (wq[rows] * INV_SQRT_HD).T.reshape(
                    DC, 128, G_HEADS, 128
                ).transpose(0, 2, 1, 3)
            )
            wkt = np.ascontiguousarray(
                wk[rows].T.reshape(DC, 128, G_HEADS, 128).transpose(0, 2, 1, 3)
            )
            wvt = np.ascontiguousarray(wv[rows].T).reshape(DC, 128, GM)
            wot = np.ascontiguousarray(wo[:, rows].T).reshape(G_HEADS, 128, SEQ)
            m = {
                "xt": xts[b], "wq": wqt, "wk": wkt, "wv": wvt, "wo": wot,
                "ce": ce, "s2": s2, "ones": ones, "mt": mt,
            }
            in_maps.append(m)

    res = run_bass_kernel_spmd(nc, in_maps, core_ids=list(range(8)), trace=trace)

    out = np.zeros((BATCH, SEQ, DIM), np.float32)
    for b in range(BATCH):
        acc = None
        for g in range(G_HEADS):
            p = res.results[b * G_HEADS + g]["out"].reshape(SEQ, DIM)
            acc = p if acc is None else acc + p
        out[b] = acc
    if trace:
        return out, res
    return out


# revision 51
# speedup vs baseline: 1.2511x; 1.0105x over previous
"""Multi-head attention (B=2, S=2048, D=2048, H=16) on 8 Trainium2 cores.

Sharding: core = batch (2) x head-group (4 heads each). Tensor-parallel on
wq/wk/wv rows + wo columns; per-core partial outputs summed on host.

Fast (mask-free) build: all matmul operands bf16 (same PE rate as fp32r in
the TRN2 cost model, half the DMA/SBUF, 2x DVE modes), one-pass QKV
projections with 16-chunk PSUM accumulation, softmax denominator via bf16
DVE accumulation + a single ones-matmul per (ic, head), h2/h3 projections
woven into the first attention blocks and the output projection woven into
the second ones so the PE never waits on the exp pipeline.

Masked build: previous two-pass fp32r kernel (correct, slower; unused when
mask is all zeros).
"""

import sys

for _p in ("/opt/trn_rl_repo",):
    if _p not in sys.path:
        sys.path.insert(0, _p)

import numpy as np
import ml_dtypes

import concourse.bass as bass
import concourse.bass_isa as bass_isa
import concourse.tile as tile
from concourse import bacc, mybir
from concourse.bass_utils import run_bass_kernel_spmd

F32 = mybir.dt.float32
F32R = mybir.dt.float32r
BF16 = mybir.dt.bfloat16
NPBF16 = ml_dtypes.bfloat16
EXP = mybir.ActivationFunctionType.Exp
ACT_COPY = mybir.ActivationFunctionType.Copy

DIM = 2048
N_HEADS = 16
HEAD_DIM = 128
BATCH = 2
SEQ = 2048
G_HEADS = 4          # heads per core
GM = G_HEADS * HEAD_DIM  # 512 output cols per core
DC = DIM // 128      # 16 contraction chunks
SC512 = SEQ // 512   # 4
SC128 = SEQ // 128   # 16
NJ2 = SC128 // 2     # 8 j-chunk pairs
INV_SQRT_HD = float(1.0 / np.sqrt(HEAD_DIM))

# even<->odd partition swap within each 32-partition group
_SWAP_MASK = [i ^ 1 for i in range(32)]


def build_fast():
    nc = bacc.Bacc("TRN2", target_bir_lowering=False, debug=False)

    xt_d = nc.dram_tensor("xt", [DC, 128, SEQ], BF16, kind="ExternalInput").ap()
    wq_d = nc.dram_tensor("wq", [G_HEADS, 128, DC * 128], BF16, kind="ExternalInput").ap()
    wk_d = nc.dram_tensor("wk", [G_HEADS, 128, DC * 128], BF16, kind="ExternalInput").ap()
    wv_d = nc.dram_tensor("wv", [128, DC * GM], BF16, kind="ExternalInput").ap()
    wo_d = nc.dram_tensor("wo", [G_HEADS, 128, SEQ], BF16, kind="ExternalInput").ap()
    ce_d = nc.dram_tensor("ce", [128, SEQ], BF16, kind="ExternalInput").ap()
    s2_d = nc.dram_tensor("s2", [128, SEQ], BF16, kind="ExternalInput").ap()
    ones_d = nc.dram_tensor("ones", [128, 1], BF16, kind="ExternalInput").ap()
    out_d = nc.dram_tensor("out", [SC128, 128, SEQ], BF16, kind="ExternalOutput").ap()

    with tile.TileContext(nc) as tc:
        consts = tc.alloc_tile_pool(name="consts", bufs=1)
        persist = tc.alloc_tile_pool(name="persist", bufs=1)
        rope_t = tc.alloc_tile_pool(name="ropet", bufs=2)

        q_t = [persist.tile([128, SEQ], BF16, tag=f"q{h}", name=f"q{h}") for h in range(G_HEADS)]
        k_t = [persist.tile([128, SEQ], BF16, tag=f"k{h}", name=f"k{h}") for h in range(G_HEADS)]
        v_t = [persist.tile([128, GM], BF16, tag=f"v{s}", name=f"v{s}") for s in range(SC128)]
        ce_t = persist.tile([128, SEQ], BF16, tag="ce", name="ce")
        s2_t = persist.tile([128, SEQ], BF16, tag="s2", name="s2")

        xf_pool = tc.alloc_tile_pool(name="xf", bufs=1)
        wqk_pool = tc.alloc_tile_pool(name="wqk", bufs=3)
        wv_pool = tc.alloc_tile_pool(name="wvp", bufs=1)
        ps_proj = tc.alloc_tile_pool(name="psproj", bufs=8, space="PSUM")

        # ---------- startup DMAs ----------
        xf = [xf_pool.tile([128, SEQ], BF16, tag=f"x{dc}", name=f"x{dc}") for dc in range(DC)]
        w0q = wqk_pool.tile([128, DC * 128], BF16, tag="w", name="w0q")
        w0k = wqk_pool.tile([128, DC * 128], BF16, tag="w", name="w0k")
        ones_t = consts.tile([128, 1], BF16, tag="ones", name="ones")
        junk_t = consts.tile([128, 512], BF16, tag="junk", name="junk")
        with tc.high_priority():
            nc.sync.dma_start(w0q[:, 0:512], wq_d[0][:, 0:512])
            nc.scalar.dma_start(w0k[:, 0:512], wk_d[0][:, 0:512])
            nc.sync.dma_start(xf[0][:, 0:1024], xt_d[0][:, 0:1024])
            nc.gpsimd.dma_start(ones_t[:], ones_d)
            # PE p-state warm-up: a few throwaway matmuls so the ramp epoch
            # starts at ~1us and the first real matmuls run at full rate
            nc.vector.memset(junk_t[:], 0.5)
            warm_ps = ps_proj.tile([128, 512], F32, tag="pp", name="warm_ps")
            for i in range(7):
                nc.tensor.matmul(warm_ps[:], junk_t[:, 0:128], junk_t[:],
                                 start=(i == 0), stop=(i == 6))
        warm_t = consts.tile([128, 1], F32, tag="warm", name="warm")
        nc.scalar.activation(out=warm_t[:], in_=ones_t[:], func=EXP)
        # first-half (xa) chunks feed G0's pass A at 256KB granularity —
        # faster than the 4-chain consumption rate, so PE never starves
        for dc in range(1, 4):
            eng = nc.scalar if dc % 2 == 1 else nc.sync
            eng.dma_start(xf[dc][:, 0:1024], xt_d[dc][:, 0:1024])
        nc.sync.dma_start(w0q[:, 512:], wq_d[0][:, 512:])
        nc.scalar.dma_start(w0k[:, 512:], wk_d[0][:, 512:])
        for dc in range(4, DC):
            eng = nc.scalar if dc % 2 == 1 else nc.sync
            eng.dma_start(xf[dc][:, 0:1024], xt_d[dc][:, 0:1024])
        for dc in range(DC):
            eng = nc.scalar if dc % 2 == 1 else nc.sync
            eng.dma_start(xf[dc][:, 1024:], xt_d[dc][:, 1024:])
        # h1 weights then rope tables — all needed right at the end of G0.
        # w1k reuses w0q's pool slot (frees at G0 end) so it must not sit
        # ahead of ce/s2 in a queue.
        w1q = wqk_pool.tile([128, DC * 128], BF16, tag="w", name="w1q")
        nc.sync.dma_start(w1q[:], wq_d[1])
        w1k = wqk_pool.tile([128, DC * 128], BF16, tag="w", name="w1k")
        nc.sync.dma_start(w1k[:], wk_d[1])
        nc.scalar.dma_start(ce_t[:], ce_d)
        nc.scalar.dma_start(s2_t[:], s2_d)

        def rope_split(ps, dst, sc, t3_pool=False):
            """ACT-copy the PSUM chain result into its bf16 slice, start the
            sin-product on GpSimd; return a closure emitting the remaining
            DVE ops (all-SBUF bf16, so they run in the 2x mode). GPSIMD must
            never touch PSUM (BIR verifier rule)."""
            sl = slice(sc * 512, (sc + 1) * 512)
            nc.scalar.activation(out=dst[:, sl], in_=ps[:], func=ACT_COPY)
            t1 = rope_t.tile([128, 512], BF16, tag="t1", name="t1")
            nc.gpsimd.tensor_mul(out=t1[:], in0=dst[:, sl], in1=s2_t[:, sl])

            def back():
                t3 = rope_t.tile([128, 512], BF16, tag="t3", name="t3")
                nc.vector.tensor_mul(out=t3[:], in0=dst[:, sl], in1=ce_t[:, sl])
                t2 = rope_t.tile([128, 512], BF16, tag="t2", name="t2")
                nc.vector.stream_shuffle(t2[:], t1[:], _SWAP_MASK)
                nc.vector.tensor_add(out=dst[:, sl], in0=t3[:], in1=t2[:])
            return back

        def rope(ps, dst, sc, t3_pool=False):
            rope_split(ps, dst, sc, t3_pool)()

        # ---------- G0: h0 q,k — two dc-major passes over column halves,
        # paced by the 256KB xa/xb feed; pass A banks free during pass B ----
        for half in range(2):
            chains0 = [("q", 2 * half), ("q", 2 * half + 1),
                       ("k", 2 * half), ("k", 2 * half + 1)]
            ps0 = [ps_proj.tile([128, 512], F32, tag="pp", name=f"pp{ci}")
                   for ci in range(len(chains0))]
            for dc in range(DC):
                for ci, (qk, sc) in enumerate(chains0):
                    w = w0q if qk == "q" else w0k
                    nc.tensor.matmul(
                        ps0[ci][:], w[:, dc * 128:(dc + 1) * 128],
                        xf[dc][:, sc * 512:(sc + 1) * 512],
                        start=(dc == 0), stop=(dc == DC - 1),
                    )
            for ci, (qk, sc) in enumerate(chains0):
                rope(ps0[ci], q_t[0] if qk == "q" else k_t[0], sc)

        # ---------- G1: h1 q,k — chain-major ----------
        for qk, w, dst in (("q", w1q, q_t[1]), ("k", w1k, k_t[1])):
            for sc in range(SC512):
                ps = ps_proj.tile([128, 512], F32, tag="pp", name="pp")
                for dc in range(DC):
                    nc.tensor.matmul(
                        ps[:], w[:, dc * 128:(dc + 1) * 128],
                        xf[dc][:, sc * 512:(sc + 1) * 512],
                        start=(dc == 0), stop=(dc == DC - 1),
                    )
                rope(ps, dst, sc)

        # ---------- V: 16 chains, ACT copies to v tiles ----------
        wvt = wv_pool.tile([128, DC * GM], BF16, tag="wv", name="wvt")
        for i in range(4):
            nc.scalar.dma_start(
                wvt[:, i * 2048:(i + 1) * 2048], wv_d[:, i * 2048:(i + 1) * 2048]
            )
        for s in range(SC128):
            ps = ps_proj.tile([128, GM], F32, tag="pp", name="pp")
            for dc in range(DC):
                nc.tensor.matmul(
                    ps[:], xf[dc][:, s * 128:(s + 1) * 128],
                    wvt[:, dc * GM:(dc + 1) * GM],
                    start=(dc == 0), stop=(dc == DC - 1),
                )
            nc.scalar.activation(out=v_t[s][:], in_=ps[:], func=ACT_COPY)
        wv_pool.release()

        # weights for the h2/h3 fill chains (k first: needed by hp1 earliest)
        w2k = wqk_pool.tile([128, DC * 128], BF16, tag="w", name="w2k")
        nc.sync.dma_start(w2k[:], wk_d[2])
        w3k = wqk_pool.tile([128, DC * 128], BF16, tag="w", name="w3k")
        nc.scalar.dma_start(w3k[:], wk_d[3])
        w2q = wqk_pool.tile([128, DC * 128], BF16, tag="w", name="w2q")
        nc.sync.dma_start(w2q[:], wq_d[2])
        w3q = wqk_pool.tile([128, DC * 128], BF16, tag="w", name="w3q")
        nc.scalar.dma_start(w3q[:], wq_d[3])

        ps_proj.release()

        # ---------- attention pools ----------
        o_pool = tc.alloc_tile_pool(name="oT", bufs=1)
        o_t = [o_pool.tile([128, SEQ], BF16, tag=f"o{h}", name=f"o{h}") for h in range(G_HEADS)]
        e_pool = tc.alloc_tile_pool(name="e", bufs=4)
        acc_pool = tc.alloc_tile_pool(name="acce", bufs=3)
        eh_pool = tc.alloc_tile_pool(name="eh", bufs=2)
        red_pool = tc.alloc_tile_pool(name="red", bufs=3)
        f_pool = tc.alloc_tile_pool(name="fp", bufs=4)
        ps_st = tc.alloc_tile_pool(name="ps_st", bufs=2, space="PSUM")
        ps_av = tc.alloc_tile_pool(name="ps_av", bufs=2, space="PSUM")
        ps_fill = tc.alloc_tile_pool(name="ps_fill", bufs=2, space="PSUM")

        wo_pool = tc.alloc_tile_pool(name="wo", bufs=1)
        wo_t = []  # tiles created after xf releases (SBUF headroom)

        def p3_mm(s, nck, psum_pool):
            ps = psum_pool.tile([128, 512], F32, tag="fill", name="psf")
            for mc in range(G_HEADS):
                nc.tensor.matmul(
                    ps[:], o_t[mc][:, s * 128:(s + 1) * 128],
                    wo_t[mc][:, nck * 512:(nck + 1) * 512],
                    start=(mc == 0), stop=(mc == G_HEADS - 1),
                )
            return ps

        def p3_out(s, nck, ps, eng, fstate, paired=True):
            # pairs of 512-col results share one [128,1024] staging tile and
            # one store, halving HWDGE issue pressure; the last tail chains
            # store unpaired so the end-of-kernel DMA drain is shorter
            if not paired:
                fo = f_pool.tile([128, 512], BF16, tag="fs", name="fs")
                if eng == 0:
                    nc.scalar.activation(out=fo[:], in_=ps[:], func=ACT_COPY)
                else:
                    nc.vector.tensor_copy(out=fo[:], in_=ps[:])
                st_eng = nc.scalar if eng else nc.sync
                st_eng.dma_start(out_d[s][:, nck * 512:(nck + 1) * 512], fo[:])
                return
            if nck % 2 == 0:
                fstate["f"] = f_pool.tile([128, 1024], BF16, tag="f", name="f")
            fo = fstate["f"]
            half = (nck % 2) * 512
            dst = fo[:, half:half + 512]
            if eng == 0:
                nc.scalar.activation(out=dst, in_=ps[:], func=ACT_COPY)
            else:
                nc.vector.tensor_copy(out=dst, in_=ps[:])
            if nck % 2 == 1:
                st_eng = nc.scalar if (nck // 2) % 2 == 0 else nc.sync
                st_eng.dma_start(out_d[s][:, (nck - 1) * 512:(nck + 1) * 512], fo[:])

        def make_g23_fill(chains, per_block=None):
            # h2/h3 projection chains, 4 matmul-groups each. The rope-front
            # consumers of chain i-1 are emitted after chain i's first group
            # so the two ps_fill banks cycle without stalling PE. per_block
            # caps matmul-groups per block so the fill spreads evenly.
            state = {"i": 0, "g": 0, "ps": None,
                     "front": None, "back": None, "blk": 0}

            def flush_front():
                if state["front"] is not None:
                    state["back"] = state["front"]()
                    state["front"] = None

            def flush_back():
                if state["back"] is not None:
                    state["back"]()
                    state["back"] = None

            def fill():
                i = state["i"]
                if i >= len(chains):
                    if state["front"] or state["back"]:
                        flush_front()
                        flush_back()
                        return True
                    return False
                if per_block is not None and state["blk"] >= per_block:
                    return True
                state["blk"] += 1
                h, qk, sc, w = chains[i]
                g = state["g"]
                if g == 0:
                    state["ps"] = ps_fill.tile([128, 512], F32, tag="fill", name="psf")
                ps = state["ps"]
                for dc in range(4 * g, 4 * g + 4):
                    nc.tensor.matmul(
                        ps[:], w[:, dc * 128:(dc + 1) * 128],
                        xf[dc][:, sc * 512:(sc + 1) * 512],
                        start=(dc == 0), stop=(dc == DC - 1),
                    )
                if g == 0:
                    flush_front()
                elif g == 1:
                    flush_back()
                elif g == 3:
                    dst = q_t[h] if qk == "q" else k_t[h]
                    state["front"] = (
                        lambda p=ps, d=dst, s_=sc: rope_split(p, d, s_, t3_pool=True)
                    )
                    state["i"] += 1
                state["g"] = (g + 1) % 4
                return True

            def new_block():
                state["blk"] = 0
            fill.new_block = new_block
            return fill

        def make_p3_fill(ic, skip=4, psum_pool=None, engs=(0, 1, 0, 1),
                         unpair_from=16):
            # chain n's copy+store lag one slot behind its matmuls
            pool = psum_pool if psum_pool is not None else ps_fill
            state = {"n": -skip, "prev": None, "f": None}

            def out_prev():
                s, nck, ps, k = state["prev"]
                p3_out(s, nck, ps, engs[k % len(engs)], state,
                       paired=(k < unpair_from))
                state["prev"] = None

            def fill():
                n = state["n"]
                if n >= 16:
                    if state["prev"] is not None:
                        out_prev()
                        return True
                    return False
                state["n"] += 1
                if n < 0:
                    return True
                s, nck = ic * 4 + n // 4, n % 4
                ps = p3_mm(s, nck, pool)
                if state["prev"] is not None:
                    out_prev()
                state["prev"] = (s, nck, ps, n)
                return True
            return fill

        def noop_fill():
            return False

        def emit_block(ha, hb, ic, fill, prev_tail):
            """One attention block for local heads (ha, hb) on query chunk ic.
            prev_tail: (eh_fns, den_fns, fin_fns) per-head closures of the
            previous block's denominator/normalize, woven in here so the PE
            never waits on them at a block seam. Returns this block's tail."""
            isl = slice(ic * 512, (ic + 1) * 512)
            acc = {}
            acc_e = {}
            e_of = {}
            for h in (ha, hb):
                acc[h] = ps_av.tile([128, 512], F32, tag="acc", name="acc")

            def emit_st(h, jc2, copy_pool=False):
                ja, jb = 2 * jc2, 2 * jc2 + 1
                st = ps_st.tile([128, 1024], F32, tag="st", name="st")
                nc.tensor.matmul(
                    st[:, 0:512], k_t[h][:, ja * 128:(ja + 1) * 128], q_t[h][:, isl],
                    start=True, stop=True,
                )
                nc.tensor.matmul(
                    st[:, 512:1024], k_t[h][:, jb * 128:(jb + 1) * 128], q_t[h][:, isl],
                    start=True, stop=True,
                )
                e = e_pool.tile([128, 1024], BF16, tag="e", name="e")
                nc.scalar.activation(out=e[:], in_=st[:], func=EXP)
                if jc2 == 0:
                    a = acc_pool.tile([128, 1024], BF16, tag="acce", name="acce")
                    acc_e[h] = a
                    # first copy off the DVE queue head so the previous
                    # block's eh/recip chain runs unblocked
                    eng = nc.gpsimd if copy_pool else nc.vector
                    eng.tensor_copy(out=a[:], in_=e[:])
                else:
                    nc.vector.tensor_add(out=acc_e[h][:], in0=acc_e[h][:], in1=e[:])
                e_of[(h, jc2)] = e

            def emit_av(h, jc2):
                ja, jb = 2 * jc2, 2 * jc2 + 1
                e = e_of.pop((h, jc2))
                nc.tensor.matmul(
                    acc[h][:], v_t[ja][:, h * 128:(h + 1) * 128], e[:, 0:512],
                    start=(jc2 == 0), stop=False,
                )
                nc.tensor.matmul(
                    acc[h][:], v_t[jb][:, h * 128:(h + 1) * 128], e[:, 512:1024],
                    start=False, stop=(jc2 == NJ2 - 1),
                )

            p_eh, p_den, p_fin = prev_tail if prev_tail else (None, None, None)
            if p_eh:
                p_eh[0]()
            emit_st(ha, 0, copy_pool=True)
            if p_eh:
                p_eh[1]()
            fill()
            if p_den:
                p_den[0]()
            emit_st(hb, 0)
            fill()
            if p_den:
                p_den[1]()
            if p_fin:
                p_fin[0]()
                p_fin[1]()
            for jc2 in range(1, NJ2):
                emit_st(ha, jc2)
                fill()
                emit_av(ha, jc2 - 1)
                emit_st(hb, jc2)
                fill()
                emit_av(hb, jc2 - 1)
            emit_av(ha, NJ2 - 1)
            fill()
            emit_av(hb, NJ2 - 1)
            fill()

            # build this block's tail closures
            eh_of = {}
            red_of = {}

            def mk_eh(h):
                def f():
                    eh = eh_pool.tile([128, 512], BF16, tag="eh", name="eh")
                    nc.vector.tensor_add(
                        out=eh[:], in0=acc_e[h][:, 0:512], in1=acc_e[h][:, 512:1024]
                    )
                    eh_of[h] = eh
                return f

            def mk_red(h):
                def f():
                    # fused partition-reduce + broadcast of the softmax
                    # denominator on the otherwise-idle GpSimd engine
                    red = red_pool.tile([128, 512], F32, tag="red", name="red")
                    nc.gpsimd.partition_all_reduce(
                        red[:], eh_of[h][:], 128, bass_isa.ReduceOp.add
                    )
                    nc.vector.reciprocal_approx_fast(out=red[:], in_=red[:])
                    red_of[h] = red
                return f

            def mk_fin(h):
                def f():
                    nc.vector.tensor_mul(
                        out=o_t[h][:, isl], in0=acc[h][:], in1=red_of[h][:]
                    )
                return f

            return ([mk_eh(ha), mk_eh(hb)],
                    [mk_red(ha), mk_red(hb)],
                    [mk_fin(ha), mk_fin(hb)])

        def run_tail(t):
            eh_fns, den_fns, fin_fns = t
            for f in eh_fns:
                f()
            for f in den_fns:
                f()
            for f in fin_fns:
                f()

        # hp0: heads 0,1 with h2/h3 projection fill; the last two chains
        # (q2/q3 sc3, not needed until hp1-ic3) spill into hp1-ic0 so that
        # block has PE filler too
        g23_chains = []
        for qk, wmap in (("k", {2: w2k, 3: w3k}), ("q", {2: w2q, 3: w3q})):
            for h in (2, 3):
                for sc in range(SC512):
                    g23_chains.append((h, qk, sc, wmap[h]))
        # only q2/q3 sc2-3 may spill into hp1-ic0: hp1-ic0/ic1 read q2/q3
        # sc0-1, which must be written before hp1 starts
        _spill = (10, 11, 14, 15)
        g23 = make_g23_fill([c for i, c in enumerate(g23_chains)
                             if i not in _spill], per_block=12)
        g23b = make_g23_fill([g23_chains[i] for i in _spill])
        tail = None
        for ic in range(SC512):
            g23.new_block()
            tail = emit_block(0, 1, ic, g23, tail)
        while g23():
            pass

        for mc in range(G_HEADS):
            w = wo_pool.tile([128, SEQ], BF16, tag=f"wo{mc}", name=f"wo{mc}")
            eng = nc.sync if mc % 2 == 0 else nc.scalar
            eng.dma_start(w[:], wo_d[mc])
            wo_t.append(w)

        # hp1: heads 2,3 with output-projection fill (one ic behind)
        fills = [g23b] + [make_p3_fill(ic) for ic in range(SC512 - 1)]
        for ic in range(SC512):
            tail = emit_block(2, 3, ic, fills[ic], tail)
            if ic < SC512 - 1:
                while fills[ic]():
                    pass

        # leftover p3(ic2) chains cover the final tail's eh->den latency
        left = fills[SC512 - 1]
        left()
        run_tail(tail)
        while left():
            pass

        ps_fill.release()
        ps_av.release()
        ps_st.release()

        ps3 = tc.alloc_tile_pool(name="ps3", bufs=4, space="PSUM")
        tail_fill = make_p3_fill(SC512 - 1, skip=0, psum_pool=ps3,
                                 engs=(0, 1, 0, 1), unpair_from=12)
        while tail_fill():
            pass
        ps3.release()

        for p in (wo_pool, f_pool, red_pool, eh_pool, acc_pool,
                  e_pool, o_pool, wqk_pool, xf_pool, rope_t, persist, consts):
            p.release()

    nc.compile()
    return nc


def build_masked():
    """Previous two-pass fp32r kernel — only used when mask is nonzero."""
    nc = bacc.Bacc("TRN2", target_bir_lowering=False, debug=False)

    xt_d = nc.dram_tensor("xt", [DC, 128, SEQ], F32R, kind="ExternalInput").ap()
    wq_d = nc.dram_tensor("wq", [DC, G_HEADS, 128, 128], F32R, kind="ExternalInput").ap()
    wk_d = nc.dram_tensor("wk", [DC, G_HEADS, 128, 128], F32R, kind="ExternalInput").ap()
    wv_d = nc.dram_tensor("wv", [DC, 128, GM], F32R, kind="ExternalInput").ap()
    wo_d = nc.dram_tensor("wo", [G_HEADS, 128, SEQ], F32R, kind="ExternalInput").ap()
    ce_d = nc.dram_tensor("ce", [128, SEQ], F32R, kind="ExternalInput").ap()
    s2_d = nc.dram_tensor("s2", [128, SEQ], F32R, kind="ExternalInput").ap()
    ones_d = nc.dram_tensor("ones", [128, 1], F32R, kind="ExternalInput").ap()
    mt_d = nc.dram_tensor("mt", [SC128, 128, SEQ], F32, kind="ExternalInput").ap()
    out_d = nc.dram_tensor("out", [SC128, 128, SEQ], F32, kind="ExternalOutput").ap()

    with tile.TileContext(nc) as tc:
        with (
            tc.tile_pool(name="persist", bufs=1) as persist,
            tc.tile_pool(name="consts", bufs=1) as consts,
        ):
            ones_t = consts.tile([128, 1], F32R, tag="ones")
            nc.sync.dma_start(ones_t[:], ones_d)
            warm_t = consts.tile([128, 1], F32, tag="warm")
            nc.scalar.activation(
                out=warm_t[:], in_=ones_t[:],
                func=EXP,
            )

            q_t = [persist.tile([128, SEQ], F32R, tag=f"q{h}", name=f"q{h}") for h in range(G_HEADS)]
            k_t = [persist.tile([128, SEQ], F32R, tag=f"k{h}", name=f"k{h}") for h in range(G_HEADS)]
            v_t = [persist.tile([128, GM], F32R, tag=f"v{s}", name=f"v{s}") for s in range(SC128)]

            with (
                tc.tile_pool(name="rope_c", bufs=1) as rope_c,
                tc.tile_pool(name="xt", bufs=8) as xt_pool,
                tc.tile_pool(name="wqk", bufs=2) as wqk_pool,
                tc.tile_pool(name="wv", bufs=1) as wv_pool,
                tc.tile_pool(name="ps1", bufs=4, space="PSUM") as ps1,
                tc.tile_pool(name="rope_t", bufs=1) as rope_t,
            ):
                ce_t = rope_c.tile([128, SEQ], F32R, tag="ce")
                s2_t = rope_c.tile([128, SEQ], F32R, tag="s2")

                def rope(t, sl):
                    t1 = rope_t.tile([128, 512], F32, tag="t1", name="t1")
                    nc.gpsimd.tensor_mul(out=t1[:], in0=t[:, sl], in1=s2_t[:, sl])
                    t2 = rope_t.tile([128, 512], F32, tag="t2", name="t2")
                    nc.vector.stream_shuffle(t2[:], t1[:], _SWAP_MASK)
                    t3 = rope_t.tile([128, 512], F32, tag="t3", name="t3")
                    nc.vector.tensor_mul(out=t3[:], in0=t[:, sl], in1=ce_t[:, sl])
                    nc.vector.tensor_add(out=t[:, sl], in0=t3[:], in1=t2[:])

                for half in range(2):
                    dcs = list(range(half * 8, half * 8 + 8))
                    prio = tc.high_priority() if half == 0 else None
                    if prio is not None:
                        prio.__enter__()
                    wt_first = wqk_pool.tile([128, 8, 128], F32R, tag="w", name="wt")
                    nc.sync.dma_start(
                        wt_first[:],
                        wq_d[dcs[0] : dcs[0] + 8, 0].rearrange("c p m -> p c m"),
                    )
                    xtsA, xtsB = [], []
                    for qi, dc in enumerate(dcs):
                        xa = xt_pool.tile([128, 1024], F32R, tag="xa", name="xa")
                        eng = nc.sync if qi % 2 == 0 else nc.scalar
                        eng.dma_start(xa[:], xt_d[dc][:, 0:1024])
                        xtsA.append(xa)
                    if prio is not None:
                        prio.__exit__(None, None, None)
                    for qi, dc in enumerate(dcs):
                        xb = xt_pool.tile([128, 1024], F32R, tag="xb", name="xb")
                        eng = nc.sync if qi % 2 == 1 else nc.scalar
                        eng.dma_start(xb[:], xt_d[dc][:, 1024:2048])
                        xtsB.append(xb)

                    def xslice(i, sl_start, width):
                        if sl_start < 1024:
                            return xtsA[i][:, sl_start : sl_start + width]
                        return xtsB[i][:, sl_start - 1024 : sl_start - 1024 + width]
                    wvt = wv_pool.tile([128, 8, GM], F32R, tag="wv", name="wvt")
                    nc.scalar.dma_start(
                        wvt[:], wv_d[dcs[0] : dcs[0] + 8].rearrange("c p m -> p c m")
                    )
                    v_next = 0

                    def emit_v(n):
                        nonlocal v_next
                        for s in range(v_next, v_next + n):
                            ps = ps1.tile([128, GM], F32, tag="ps", name="ps")
                            for i in range(8):
                                nc.tensor.matmul(
                                    ps[:], xslice(i, s * 128, 128), wvt[:, i, :],
                                    start=(i == 0), stop=(i == 7),
                                )
                            if half == 0:
                                nc.vector.tensor_copy(out=v_t[s][:], in_=ps[:])
                            else:
                                nc.vector.tensor_add(
                                    out=v_t[s][:], in0=ps[:], in1=v_t[s][:]
                                )
                        v_next += n

                    for h in range(G_HEADS):
                        for wi, (wd, dst) in enumerate(((wq_d, q_t[h]), (wk_d, k_t[h]))):
                            if h == 0 and wi == 0:
                                wt = wt_first
                            else:
                                wt = wqk_pool.tile([128, 8, 128], F32R, tag="w", name="wt")
                                nc.sync.dma_start(
                                    wt[:],
                                    wd[dcs[0] : dcs[0] + 8, h].rearrange("c p m -> p c m"),
                                )
                            for sc in range(SC512):
                                ps = ps1.tile([128, 512], F32, tag="ps", name="ps")
                                sl = bass.ts(sc, 512)
                                for i in range(8):
                                    nc.tensor.matmul(
                                        ps[:], wt[:, i, :], xslice(i, sc * 512, 512),
                                        start=(i == 0), stop=(i == 7),
                                    )
                                if half == 0:
                                    nc.vector.tensor_copy(out=dst[:, sl], in_=ps[:])
                                else:
                                    nc.vector.tensor_add(
                                        out=dst[:, sl], in0=ps[:], in1=dst[:, sl]
                                    )
                                    rope(dst, sl)
                        if (half == 0 and h >= 2) or half == 1:
                            emit_v({0: 8, 1: 4}[half])
                        if half == 0 and h == G_HEADS - 1:
                            nc.scalar.dma_start(ce_t[:], ce_d)
                            nc.scalar.dma_start(s2_t[:], s2_d)

            o_pool = tc.alloc_tile_pool(name="oT", bufs=1)
            o_t = [o_pool.tile([128, SEQ], F32R, tag=f"o{h}", name=f"o{h}") for h in range(G_HEADS)]
            wo_pool = tc.alloc_tile_pool(name="wo", bufs=1)
            wo_t = []
            for mc in range(G_HEADS):
                w = wo_pool.tile([128, SEQ], F32R, tag=f"wo{mc}", name=f"wo{mc}")
                nc.sync.dma_start(w[:], wo_d[mc])
                wo_t.append(w)
            with (
                tc.tile_pool(name="est", bufs=5) as est_pool,
                tc.tile_pool(name="nrm", bufs=3) as nrm_pool,
                tc.tile_pool(name="ps_st", bufs=2, space="PSUM") as ps_st,
                tc.tile_pool(name="ps_av", bufs=2, space="PSUM") as ps_av,
                tc.tile_pool(name="ps_dn", bufs=2, space="PSUM") as ps_dn,
            ):
                mask_pool = tc.alloc_tile_pool(name="mask", bufs=2)

                for ic in range(SC512):
                    isl = bass.ts(ic, 512)
                    for hp in range(G_HEADS // 2):
                        heads = (2 * hp, 2 * hp + 1)
                        acc = {}
                        den = {}
                        e_of = {}
                        m_of = {}
                        for h in heads:
                            acc[h] = ps_av.tile([128, 512], F32, tag="acc", name="acc")
                            den[h] = ps_dn.tile([1, 512], F32, tag="den", name="den")

                        def emit_st(h, jc2):
                            ja, jb = 2 * jc2, 2 * jc2 + 1
                            st = ps_st.tile([128, 1024], F32, tag="st", name="st")
                            nc.tensor.matmul(
                                st[:, 0:512],
                                k_t[h][:, bass.ts(ja, 128)], q_t[h][:, isl],
                                start=True, stop=True,
                            )
                            nc.tensor.matmul(
                                st[:, 512:1024],
                                k_t[h][:, bass.ts(jb, 128)], q_t[h][:, isl],
                                start=True, stop=True,
                            )
                            e = est_pool.tile([128, 1024], F32R, tag="e", name="e")
                            if jc2 not in m_of:
                                mtl = mask_pool.tile(
                                    [128, 1024], F32, tag="m", name="mtl"
                                )
                                nc.sync.dma_start(mtl[:, 0:512], mt_d[ja, :, isl])
                                nc.sync.dma_start(mtl[:, 512:1024], mt_d[jb, :, isl])
                                m_of[jc2] = mtl
                            nc.vector.tensor_add(
                                out=e[:], in0=st[:], in1=m_of[jc2][:]
                            )
                            nc.scalar.activation(
                                out=e[:], in_=e[:],
                                func=EXP,
                            )
                            eh = est_pool.tile([128, 512], F32R, tag="eh", name="eh", bufs=3)
                            nc.vector.tensor_add(
                                out=eh[:], in0=e[:, 0:512], in1=e[:, 512:1024]
                            )
                            e_of[h] = (e, eh)

                        def emit_denav(h, jc2):
                            ja, jb = 2 * jc2, 2 * jc2 + 1
                            e, eh = e_of[h]
                            last = jc2 == SC128 // 2 - 1
                            nc.tensor.matmul(
                                den[h][:], ones_t[:], eh[:],
                                start=(jc2 == 0), stop=last,
                            )
                            nc.tensor.matmul(
                                acc[h][:], v_t[ja][:, bass.ts(h, 128)], e[:, 0:512],
                                start=(jc2 == 0), stop=False,
                            )
                            nc.tensor.matmul(
                                acc[h][:], v_t[jb][:, bass.ts(h, 128)], e[:, 512:1024],
                                start=False, stop=last,
                            )

                        h0, h1 = heads
                        emit_st(h0, 0)
                        for jc2 in range(NJ2):
                            if jc2 > 0:
                                emit_st(h0, jc2)
                                emit_denav(h1, jc2 - 1)
                            emit_st(h1, jc2)
                            emit_denav(h0, jc2)
                        emit_denav(h1, NJ2 - 1)

                        for h in heads:
                            rec = nrm_pool.tile([1, 512], F32, tag="rec", name="rec")
                            nc.vector.reciprocal_approx_fast(out=rec[:], in_=den[h][:])
                            bc = nrm_pool.tile([128, 512], F32, tag="bc", name="bc")
                            nc.gpsimd.partition_broadcast(bc[:], rec[:])
                            nc.vector.tensor_mul(
                                out=o_t[h][:, isl], in0=acc[h][:], in1=bc[:]
                            )
                mask_pool.release()

            with (
                tc.tile_pool(name="fin", bufs=10) as fin_pool,
                tc.tile_pool(name="ps3", bufs=8, space="PSUM") as ps3,
            ):
                for s in range(SC128):
                    ssl = bass.ts(s, 128)
                    for nck in range(SC512):
                        nsl = bass.ts(nck, 512)
                        ps = ps3.tile([128, 512], F32, tag="ps3", name="ps3")
                        for mc in range(G_HEADS):
                            nc.tensor.matmul(
                                ps[:], o_t[mc][:, ssl], wo_t[mc][:, nsl],
                                start=(mc == 0), stop=(mc == G_HEADS - 1),
                            )
                        f = fin_pool.tile([128, 512], F32, tag="f", name="f")
                        nc.vector.tensor_copy(out=f[:], in_=ps[:])
                        nc.sync.dma_start(out_d[s, :, nsl], f[:])
            wo_pool.release()
            o_pool.release()

    nc.compile()
    return nc


_CACHE = {}


def _get_nc(with_mask: bool):
    if with_mask not in _CACHE:
        _CACHE[with_mask] = build_masked() if with_mask else build_fast()
    return _CACHE[with_mask]


def kernel(in_token, freqs_cos, freqs_sin, mask, wq, wk, wv, wo):
    return _run(in_token, freqs_cos, freqs_sin, mask, wq, wk, wv, wo)


def run_traced(in_token, freqs_cos, freqs_sin, mask, wq, wk, wv, wo):
    """Test-only: run with NTFF tracing, return (output, BassKernelResults)."""
    return _run(in_token, freqs_cos, freqs_sin, mask, wq, wk, wv, wo, trace=True)


def _run(in_token, freqs_cos, freqs_sin, mask, wq, wk, wv, wo, trace=False):
    in_token = np.ascontiguousarray(np.asarray(in_token, dtype=np.float32))
    freqs_cos = np.asarray(freqs_cos, dtype=np.float32)
    freqs_sin = np.asarray(freqs_sin, dtype=np.float32)
    mask = np.asarray(mask, dtype=np.float32)
    wq = np.asarray(wq, dtype=np.float32)
    wk = np.asarray(wk, dtype=np.float32)
    wv = np.asarray(wv, dtype=np.float32)
    wo = np.asarray(wo, dtype=np.float32)

    with_mask = bool(np.any(mask))
    nc = _get_nc(with_mask)

    if with_mask:
        return _run_masked(nc, in_token, freqs_cos, freqs_sin, mask,
                           wq, wk, wv, wo, trace)

    ce = np.repeat(freqs_cos.T, 2, axis=0).astype(NPBF16)  # (128, S)
    s2 = np.empty((HEAD_DIM, SEQ), np.float32)
    s2[0::2] = freqs_sin.T
    s2[1::2] = -freqs_sin.T
    s2 = s2.astype(NPBF16)
    ones = np.ones((128, 1), NPBF16)

    in_maps = []
    xts = [
        np.ascontiguousarray(in_token[b].T).reshape(DC, 128, SEQ).astype(NPBF16)
        for b in range(BATCH)
    ]
    for b in range(BATCH):
        for g in range(G_HEADS):
            rows = slice(g * GM, (g + 1) * GM)
            wqt = np.ascontiguousarray(
                (wq[rows] * INV_SQRT_HD).T.reshape(DC, 128, G_HEADS, 128)
                .transpose(2, 1, 0, 3).reshape(G_HEADS, 128, DC * 128)
            ).astype(NPBF16)
            wkt = np.ascontiguousarray(
                wk[rows].T.reshape(DC, 128, G_HEADS, 128)
                .transpose(2, 1, 0, 3).reshape(G_HEADS, 128, DC * 128)
            ).astype(NPBF16)
            wvt = np.ascontiguousarray(
                wv[rows].T.reshape(DC, 128, GM).transpose(1, 0, 2)
                .reshape(128, DC * GM)
            ).astype(NPBF16)
            wot = np.ascontiguousarray(wo[:, rows].T).reshape(
                G_HEADS, 128, SEQ
            ).astype(NPBF16)
            m = {
                "xt": xts[b], "wq": wqt, "wk": wkt, "wv": wvt, "wo": wot,
                "ce": ce, "s2": s2, "ones": ones,
            }
            in_maps.append(m)

    res = run_bass_kernel_spmd(nc, in_maps, core_ids=list(range(8)), trace=trace)

    out = np.zeros((BATCH, SEQ, DIM), np.float32)
    for b in range(BATCH):
        acc = None
        for g in range(G_HEADS):
            p = res.results[b * G_HEADS + g]["out"].astype(np.float32).reshape(SEQ, DIM)
            acc = p if acc is None else acc + p
        out[b] = acc
    if trace:
        return out, res
    return out


def _run_masked(nc, in_token, freqs_cos, freqs_sin, mask, wq, wk, wv, wo, trace):
    ce = np.repeat(freqs_cos.T, 2, axis=0).astype(np.float32)  # (128, S)
    s2 = np.empty((HEAD_DIM, SEQ), np.float32)
    s2[0::2] = freqs_sin.T
    s2[1::2] = -freqs_sin.T
    ones = np.ones((128, 1), np.float32)
    mt = np.ascontiguousarray(mask.T).reshape(SC128, 128, SEQ)

    in_maps = []
    xts = [
        np.ascontiguousarray(in_token[b].T).reshape(DC, 128, SEQ)
        for b in range(BATCH)
    ]
    for b in range(BATCH):
        for g in range(G_HEADS):
            rows = slice(g * GM, (g + 1) * GM)
            wqt = np.ascontiguousarray(
                (wq[rows] * INV_SQRT_HD).T.reshape(
                    DC, 128, G_HEADS, 128
                ).transpose(0, 2, 1, 3)
            )
            wkt = np.ascontiguousarray(
                wk[rows].T.reshape(DC, 128, G_HEADS, 128).transpose(0, 2, 1, 3)
            )
            wvt = np.ascontiguousarray(wv[rows].T).reshape(DC, 128, GM)
            wot = np.ascontiguousarray(wo[:, rows].T).reshape(G_HEADS, 128, SEQ)
            m = {
                "xt": xts[b], "wq": wqt, "wk": wkt, "wv": wvt, "wo": wot,
                "ce": ce, "s2": s2, "ones": ones, "mt": mt,
            }
            in_maps.append(m)

    res = run_bass_kernel_spmd(nc, in_maps, core_ids=list(range(8)), trace=trace)

    out = np.zeros((BATCH, SEQ, DIM), np.float32)
    for b in range(BATCH):
        acc = None
        for g in range(G_HEADS):
            p = res.results[b * G_HEADS + g]["out"].reshape(SEQ, DIM)
            acc = p if acc is None else acc + p
        out[b] = acc
    if trace:
        return out, res
    return out
